# revision 1
# baseline (speedup 1.0000x reference)
"""TRN2 Bass kernel for nn_LinearAttention (B=4, L=4096, D=1024, H=16, dh=64).

Strategy: shard (batch, head-group) across 8 cores — core c handles batch c//2,
heads (c%2)*8..(c%2)*8+8. Zero cross-core communication; the two half-head
partial output projections per batch are summed on the host during unshard.

Per-core pipeline (super-tiles of 512 tokens, scan chunks of 128):
  1. QKV projection (f32r matmuls, full-rate at moving dim 512): q,k produced
     transposed (feat, tok) with phi=elu+1 fused on the PSUM->SBUF path
     (phi = relu(x) + exp(min(x,0)), exact); v produced (tok, feat) with a
     ones-column appended so the denominator rides along in the matmuls.
  2. Chunked linear attention: per head-chunk A^T = K^T Q (bf16, masked
     causal-inclusive), O = A^T_m^T V' + Q S'_prev (second matmul fp32),
     state S' += K^T V' kept in full fp32 (S' holds [S | z]).
     den = max(O[:,64], eps), out = O[:,:64] * recip(den).
     PSUM tiles are shared across heads with a single accumulation group per
     bank (start=True only on the bank's first matmul) - interleaved
     accumulation groups in one bank crash on HW.
  3. PE-transpose per-chunk outputs to (feat, tok), output projection with
     f32r matmuls, bounce PSUM->SBUF, DMA the transposed partial to HBM.
Dtype notes: f32r ~1.9e-4 matmul rel err (full rate at N>=256), bf16 ~3e-3;
scan state stays fp32 so error does not accumulate over the 32-chunk scan.
End-to-end rel err vs fp32 reference: 1.8e-3.
"""
import sys

sys.path.insert(0, "/opt/trn_rl_repo")
import numpy as np

D = 1024
L = 4096
B = 4
H = 16
DH = 64
FPC = 512          # features per core (8 heads x 64)
C = 128            # scan chunk
ST = 512           # super-tile tokens
NCH = ST // C      # 4
NST = L // ST      # 8
EPS = 1e-6

_CACHE = {}
import os
ATT_BF16 = os.environ.get("ATT_BF16", "1") == "1"


def _build_nc(att_bf16=ATT_BF16):
    import concourse.bacc as bacc
    import concourse.mybir as mybir
    import concourse.tile as tile

    dt = mybir.dt
    f32, f32r = dt.float32, dt.float32r
    adt = dt.bfloat16 if att_bf16 else f32
    Alu = mybir.AluOpType
    Act = mybir.ActivationFunctionType

    nc = bacc.Bacc("TRN2", target_bir_lowering=False, debug=True)

    xT_d = nc.dram_tensor("xT", [D, L], f32r, kind="ExternalInput")
    w1qk_d = nc.dram_tensor("w1qk", [D, 2 * FPC], f32r, kind="ExternalInput")
    w1v_d = nc.dram_tensor("w1v", [D, FPC], f32r, kind="ExternalInput")
    w2_d = nc.dram_tensor("w2", [FPC, D], f32r, kind="ExternalInput")
    maskT_d = nc.dram_tensor("maskT", [C, C], f32, kind="ExternalInput")
    id128_d = nc.dram_tensor("id128", [128, 128], f32, kind="ExternalInput")
    id64_d = nc.dram_tensor("id64", [128, DH], f32, kind="ExternalInput")
    outT_d = nc.dram_tensor("outT", [D, L], f32, kind="ExternalOutput")

    with tile.TileContext(nc) as tc:
        with (
            tc.tile_pool(name="wp", bufs=1) as wp,
            tc.tile_pool(name="xp", bufs=2) as xp,
            tc.tile_pool(name="qp", bufs=2) as qp,
            tc.tile_pool(name="vp", bufs=2) as vp,
            tc.tile_pool(name="scr", bufs=3) as scr,
            tc.tile_pool(name="atsp", bufs=4) as atsp,
            tc.tile_pool(name="kp", bufs=4) as kp,
            tc.tile_pool(name="dp", bufs=8) as dp,
            tc.tile_pool(name="op", bufs=6) as op,
            tc.tile_pool(name="osp", bufs=2) as osp,
            tc.tile_pool(name="szp", bufs=1) as szp,
            tc.tile_pool(name="projp", bufs=2, space="PSUM") as projp,
            tc.tile_pool(name="atp", bufs=2, space="PSUM") as atp,
            tc.tile_pool(name="opp", bufs=2, space="PSUM") as opp,
            tc.tile_pool(name="miscp", bufs=1, space="PSUM") as miscp,
            tc.tile_pool(name="szkvp", bufs=1, space="PSUM") as szkvp,
        ):
            # ---- resident constants (spread across DMA issuers; xT st0 goes first) ----
            w1qk = wp.tile([128, 8, 2 * FPC], f32r, tag="w1qk")
            for cc in range(2):
                nc.sync.dma_start(
                    w1qk[:, 4 * cc : 4 * cc + 4, :],
                    w1qk_d[:].rearrange("(c p) f -> p c f", p=128)[
                        :, 4 * cc : 4 * cc + 4, :
                    ],
                )
            w1v = wp.tile([128, 8, FPC], f32r, tag="w1v")
            nc.scalar.dma_start(w1v[:], w1v_d[:].rearrange("(c p) f -> p c f", p=128))
            w2 = wp.tile([128, 4, D], f32r, tag="w2")
            nc.scalar.dma_start(w2[:], w2_d[:].rearrange("(g p) f -> p g f", p=128))
            maskT = wp.tile([C, C], f32, tag="maskT")
            nc.gpsimd.dma_start(maskT[:], maskT_d[:])
            id128 = wp.tile([128, 128], f32, tag="id128")
            nc.gpsimd.dma_start(id128[:], id128_d[:])
            id64 = wp.tile([128, DH], f32, tag="id64")
            nc.gpsimd.dma_start(id64[:], id64_d[:])
            id64a = wp.tile([128, DH], adt, tag="id64a")
            nc.vector.tensor_copy(id64a[:], id64[:])
            id128a = wp.tile([128, 128], adt, tag="id128a")
            nc.vector.tensor_copy(id128a[:], id128[:])
            id128r = wp.tile([128, 128], f32r, tag="id128r")
            nc.vector.tensor_copy(id128r[:], id128[:])

            # persistent scan state: [S | z] per head, packed 2 heads/partition-half
            Sz = szp.tile([128, 4, DH + 1], f32, tag="Sz")
            nc.vector.memset(Sz[:], 0.0)

            for st in range(NST):
                t0 = st * ST
                xT = xp.tile([128, 8, ST], f32r, tag="xT")
                xsrc = xT_d[:, t0 : t0 + ST].rearrange("(c p) t -> p c t", p=128)
                nc.sync.dma_start(xT[:, 0:4, :], xsrc[:, 0:4, :])
                nc.sync.dma_start(xT[:, 4:8, :], xsrc[:, 4:8, :])

                # ---- stage 1a: q,k transposed (feat, tok) with phi fused ----
                qT32 = qp.tile([128, 4, ST], f32, tag="qT32")
                if att_bf16:
                    qTb = qp.tile([128, 4, ST], adt, tag="qTb")
                else:
                    qTb = qT32
                kTb = qp.tile([128, 4, ST], adt, tag="kTb")
                for fc in range(8):
                    pq = projp.tile([128, ST], f32, tag="proj")
                    for dc in range(8):
                        nc.tensor.matmul(
                            pq[:],
                            w1qk[:, dc, fc * 128 : (fc + 1) * 128],
                            xT[:, dc, :],
                            start=(dc == 0),
                            stop=(dc == 7),
                        )
                    tneg = scr.tile([128, ST], f32, tag="tneg")
                    nc.scalar.activation(tneg[:], pq[:], Act.Relu, scale=-1.0)
                    texp = scr.tile([128, ST], f32, tag="texp")
                    nc.scalar.activation(texp[:], tneg[:], Act.Exp, scale=-1.0)
                    # phi(x) = relu(x) + exp(min(x,0))
                    if fc < 4:
                        nc.vector.scalar_tensor_tensor(
                            qT32[:, fc, :], pq[:], 0.0, texp[:], Alu.max, Alu.add
                        )
                        if att_bf16:
                            nc.vector.tensor_copy(qTb[:, fc, :], qT32[:, fc, :])
                    else:
                        nc.vector.scalar_tensor_tensor(
                            kTb[:, fc - 4, :], pq[:], 0.0, texp[:], Alu.max, Alu.add
                        )

                # ---- stage 1b: v in (tok, feat) + ones column ----
                v1 = vp.tile([128, NCH, 8, DH + 1], adt, tag="v1")
                nc.vector.memset(v1[:, :, :, DH], 1.0)
                for tcc in range(NCH):
                    pv = projp.tile([128, FPC], f32, tag="proj")
                    for dc in range(8):
                        nc.tensor.matmul(
                            pv[:],
                            xT[:, dc, tcc * 128 : (tcc + 1) * 128],
                            w1v[:, dc, :],
                            start=(dc == 0),
                            stop=(dc == 7),
                        )
                    nc.scalar.copy(
                        v1[:, tcc, :, 0:DH], pv[:].rearrange("p (h e) -> p h e", e=DH)
                    )

                # ---- stage 2: chunked linear attention scan ----
                outT_sb = osp.tile([128, 4, ST], f32r, tag="outT")
                out_cs = []
                for tcc in range(NCH):
                    out_c = op.tile([128, 8, DH], f32r, tag="out_c")
                    out_cs.append(out_c)

                    # k transposed to (tok, dh): 4 pair-transposes share one
                    # PSUM bank (single accumulation group), one bulk copy out
                    ksb = kp.tile([128, 8, DH], adt, tag="ksb")
                    ktr = miscp.tile([128, 4, 128], adt, tag="misc")
                    for j in range(4):
                        kTj = kTb[:, j, tcc * 128 : (tcc + 1) * 128]
                        nc.tensor.matmul(
                            ktr[:, j, :], kTj, id128a[:],
                            is_transpose=True,
                            start=(j == 0), stop=(j == 3),
                            skip_group_check=True,
                        )
                    nc.scalar.copy(
                        ksb[:], ktr[:].rearrange("p j (h e) -> p (j h) e", e=DH)
                    )

                    # per-head: A^T = K^T Q, mask, O = A^T_m^T V' + Q S'
                    # 4 heads share one PSUM bank; ONE accumulation group per bank
                    # (start=True only on the bank's first matmul - per-element
                    # has_written gives overwrite-on-first-touch for later slices)
                    Og = []
                    for j in range(2):
                        Oj = opp.tile([128, 4 * (DH + 1)], f32, tag="O4")
                        Og.append(Oj)
                    for h in range(8):
                        po = 64 * (h % 2)
                        fq = h // 2
                        qTh32 = qT32[po : po + 64, fq, tcc * 128 : (tcc + 1) * 128]
                        qThb = qTb[po : po + 64, fq, tcc * 128 : (tcc + 1) * 128]
                        kTh = kTb[po : po + 64, fq, tcc * 128 : (tcc + 1) * 128]
                        v1h = v1[:, tcc, h, :]
                        Szh = Sz[po : po + 64, h // 2, :]

                        ATp = atp.tile([C, C], f32, tag="ATp")
                        nc.tensor.matmul(ATp[:], kTh, qThb, start=True, stop=True)
                        ATs = atsp.tile([C, C], adt, tag="ATs")
                        nc.vector.tensor_tensor(ATs[:], ATp[:], maskT[:], Alu.mult)

                        Oh = Og[h // 4][:, (h % 4) * (DH + 1) : (h % 4 + 1) * (DH + 1)]
                        nc.tensor.matmul(
                            Oh, ATs[:], v1h,
                            start=(h % 4 == 0), stop=False,
                            skip_group_check=True,
                        )
                        nc.tensor.matmul(
                            Oh, qTh32, Szh,
                            start=False, stop=(h % 4 == 3),
                            skip_group_check=True,
                        )

                    # state update: head pairs stacked in one PSUM tile
                    for j in range(4):
                        Szkv = szkvp.tile([128, DH + 1], f32, tag="szkv")
                        nc.tensor.matmul(
                            Szkv[0:64, :], ksb[:, 2 * j, :], v1[:, tcc, 2 * j, :],
                            start=True, stop=False, skip_group_check=True,
                        )
                        nc.tensor.matmul(
                            Szkv[64:128, :], ksb[:, 2 * j + 1, :],
                            v1[:, tcc, 2 * j + 1, :],
                            start=True, stop=True, skip_group_check=True,
                        )
                        nc.vector.tensor_tensor(
                            Sz[:, j, :], Sz[:, j, :], Szkv[:], Alu.add
                        )

                    # batched normalization: den/recip/div for 4 heads at once
                    for j in range(2):
                        Ov = Og[j][:].rearrange("p (h e) -> p h e", e=DH + 1)
                        den = dp.tile([C, 4], f32, tag="den")
                        nc.vector.tensor_scalar(
                            den[:], Ov[:, :, DH], EPS, None, Alu.max
                        )
                        rden = dp.tile([C, 4], f32, tag="rden")
                        nc.vector.reciprocal(rden[:], den[:])
                        nc.vector.tensor_tensor(
                            out_c[:, 4 * j : 4 * j + 4, :],
                            Ov[:, :, 0:DH],
                            rden[:].unsqueeze(2).broadcast_to([C, 4, DH]),
                            Alu.mult,
                        )

                # transpose chunk outputs to (feat, tok) at super-tile end
                for tcc in range(NCH):
                    oc_flat = out_cs[tcc][:].rearrange("p h e -> p (h e)")
                    trp = miscp.tile([128, 4, C], f32r, tag="misc")
                    for g in range(4):
                        nc.tensor.matmul(
                            trp[:, g, :], oc_flat[:, g * 128 : (g + 1) * 128],
                            id128r[:],
                            is_transpose=True,
                            start=(g == 0), stop=(g == 3),
                            skip_group_check=True,
                        )
                    nc.scalar.copy(
                        outT_sb[:, :, tcc * 128 : (tcc + 1) * 128], trp[:]
                    )

                # ---- stage 3: output projection (partial, transposed) ----
                for oc in range(8):
                    po3 = projp.tile([128, ST], f32, tag="proj")
                    for g in range(4):
                        nc.tensor.matmul(
                            po3[:],
                            w2[:, g, oc * 128 : (oc + 1) * 128],
                            outT_sb[:, g, :],
                            start=(g == 0),
                            stop=(g == 3),
                        )
                    o3sb = scr.tile([128, ST], f32, tag="o3sb")
                    nc.scalar.copy(o3sb[:], po3[:])
                    nc.sync.dma_start(
                        outT_d[oc * 128 : (oc + 1) * 128, t0 : t0 + ST], o3sb[:]
                    )

    nc.finalize()
    return nc


def _get_nc():
    key = ("nc", ATT_BF16)
    if key not in _CACHE:
        _CACHE[key] = _build_nc(ATT_BF16)
    return _CACHE[key]


def _make_in_maps(x, w_qkv, w_out):
    maskT = np.triu(np.ones((C, C), np.float32))  # maskT[j,i] = 1 if j <= i
    id128 = np.eye(128, dtype=np.float32)
    id64 = np.tile(np.eye(DH, dtype=np.float32), (2, 1))
    in_maps = []
    for core in range(8):
        b, g = core // 2, core % 2
        xT = np.ascontiguousarray(x[b].T)
        w1qk = np.ascontiguousarray(
            np.concatenate(
                [
                    w_qkv[512 * g : 512 * (g + 1)],
                    w_qkv[1024 + 512 * g : 1024 + 512 * (g + 1)],
                ],
                axis=0,
            ).T
        )
        w1v = np.ascontiguousarray(w_qkv[2048 + 512 * g : 2048 + 512 * (g + 1)].T)
        w2 = np.ascontiguousarray(w_out[:, 512 * g : 512 * (g + 1)].T)
        in_maps.append(
            {
                "xT": xT,
                "w1qk": w1qk,
                "w1v": w1v,
                "w2": w2,
                "maskT": maskT,
                "id128": id128,
                "id64": id64,
            }
        )
    return in_maps


def _run(inputs, trace=False):
    from concourse.bass_utils import run_bass_kernel_spmd

    nc = _get_nc()
    in_maps = _make_in_maps(inputs["x"], inputs["w_qkv"], inputs["w_out"])
    res = run_bass_kernel_spmd(nc, in_maps, core_ids=list(range(8)), trace=trace)
    out = np.empty((B, L, D), np.float32)
    for b in range(B):
        p0 = res.results[2 * b]["outT"]
        p1 = res.results[2 * b + 1]["outT"]
        out[b] = (p0 + p1).T
    return out, res


def kernel(x, w_qkv, w_out):
    out, _ = _run({"x": x, "w_qkv": w_qkv, "w_out": w_out})
    return out



# revision 14
# speedup vs baseline: 1.7149x; 1.7149x over previous
"""TRN2 Bass kernel for nn_LinearAttention (B=4, L=4096, D=1024, H=16, dh=64).

Sharding: core c = batch c//2, heads (c%2)*8..+8. Zero cross-core comms; the
two half-head partial output projections per batch are summed on the host.

Per-core pipeline (super-tiles of 512 tokens, scan chunks of 128):
  1. QKV projection:
     - q/k: fp8e4 DoubleRow matmuls (0.5 cyc/row, K=256 per call). Weights are
       scaled x32 on the host (plain w_qkv values sit in e4m3's subnormal
       range and quantize at 6-50% error; x32 moves them to normals). A bias
       ones-row (weight 32) makes PSUM hold 32(z+1); phi is exact in 2 passes
       via 32*phi(z) = max(min(32*e^z, 32), 32z+32):
       Act Exp(scale=1/32, bias=ln32-1) then one DVE scalar_tensor_tensor.
       The x32 on q/k cancels between numerator and denominator.
     - v: 3-term compensated fp8 DoubleRow (x_hi w_hi + x_hi w_lo + x_lo w_hi)
       giving ~bf16 accuracy at 0.75x bf16 matmul cost; the copy to bf16
       applies 1/128 (un-scales the x32 and pre-divides v by 4 for range).
  2. Chunked linear attention, bf16 operands / f32 PSUM:
     A^T = K^T Q per head (heads grouped by partition parity per PSUM bank --
     a PE tile-row config may not change within a bank) -> masked-copy (DVE,
     doubles as the PSUM->SBUF move) -> O = A_m V + Q S8, all 8 heads in one
     bank; den via tiny matmuls (masked-A^T . ones/32 + q . z8); state
     [S | z/32] lives PERSISTENTLY in one PSUM bank (start only at chunk 0)
     with a per-chunk bf16 snapshot for the next chunk's Q S / q.z reads.
     den' = max(den, eps/32)*8 restores every scale factor exactly.
  3. PE-transpose o (bf16) -> output projection in bf16 -> bf16 partials
     DMA'd out; the host upcasts and sums core pairs in f32.

The emission order is software-pipelined: attention chunks of super-tile s
are interleaved with the projection of super-tile s+1 so the PE never waits
on the phi-combine consumer chain (engines execute their streams in order).
"""
import sys

sys.path.insert(0, "/opt/trn_rl_repo")
import numpy as np
import ml_dtypes

F8 = ml_dtypes.float8_e4m3
BFNP = ml_dtypes.bfloat16

D = 1024
L = 4096
B = 4
H = 16
DH = 64
FPC = 512          # features per core (8 heads x 64)
C = 128            # scan chunk
ST = 512           # super-tile tokens
NCH = ST // C      # 4
NST = L // ST      # 8
EPS = 1e-6

_CACHE = {}


def _build_nc():
    import concourse.bacc as bacc
    import concourse.mybir as mybir
    import concourse.tile as tile

    dt = mybir.dt
    f32, fp8, bf16 = dt.float32, dt.float8e4, dt.bfloat16
    Alu = mybir.AluOpType
    Act = mybir.ActivationFunctionType
    PM = mybir.MatmulPerfMode

    nc = bacc.Bacc("TRN2", target_bir_lowering=False, debug=True)

    xhi_d = nc.dram_tensor("xhi", [128, 4, 2, L], fp8, kind="ExternalInput")
    xlo_d = nc.dram_tensor("xlo", [128, 4, 2, L], fp8, kind="ExternalInput")
    wqk_d = nc.dram_tensor("wqk", [128, 8, 4, 2, 128], fp8, kind="ExternalInput")
    wvh_d = nc.dram_tensor("wvh", [128, 4, 2, FPC], fp8, kind="ExternalInput")
    wvl_d = nc.dram_tensor("wvl", [128, 4, 2, FPC], fp8, kind="ExternalInput")
    w2_d = nc.dram_tensor("w2", [128, 4, 8, 128], bf16, kind="ExternalInput")
    maskT_d = nc.dram_tensor("maskT", [C, C], f32, kind="ExternalInput")
    id128_d = nc.dram_tensor("id128", [128, 128], bf16, kind="ExternalInput")
    outT_d = nc.dram_tensor("outT", [D, L], bf16, kind="ExternalOutput")

    from contextlib import ExitStack

    with tile.TileContext(nc) as tc, ExitStack() as es:
        if True:
            wp = es.enter_context(tc.tile_pool(name="wp", bufs=1))
            xp = es.enter_context(tc.tile_pool(name="xp", bufs=2))
            qp = es.enter_context(tc.tile_pool(name="qp", bufs=4))
            scr = es.enter_context(tc.tile_pool(name="scr", bufs=3))
            vp = es.enter_context(tc.tile_pool(name="vp", bufs=2))
            kp = es.enter_context(tc.tile_pool(name="kp", bufs=3))
            atsp = es.enter_context(tc.tile_pool(name="atsp", bufs=3))
            szsp = es.enter_context(tc.tile_pool(name="szsp", bufs=3))
            dp = es.enter_context(tc.tile_pool(name="dp", bufs=4))
            ocp = es.enter_context(tc.tile_pool(name="ocp", bufs=3))
            osp = es.enter_context(tc.tile_pool(name="osp", bufs=2))
            o3p = es.enter_context(tc.tile_pool(name="o3p", bufs=2))
            projp = es.enter_context(tc.tile_pool(name="projp", bufs=2, space="PSUM"))
            atp = es.enter_context(tc.tile_pool(name="atp", bufs=2, space="PSUM"))
            obp = es.enter_context(tc.tile_pool(name="obp", bufs=1, space="PSUM"))
            szp = es.enter_context(tc.tile_pool(name="szp", bufs=1, space="PSUM"))
            miscp = es.enter_context(tc.tile_pool(name="miscp", bufs=1, space="PSUM"))
            trpp = es.enter_context(tc.tile_pool(name="trpp", bufs=1, space="PSUM"))

            # ---- resident constants / weights ----
            wqk = wp.tile([128, 8, 4, 2, 128], fp8, tag="wqk")
            nc.scalar.dma_start(wqk[:], wqk_d[:])
            wvh = wp.tile([128, 4, 2, FPC], fp8, tag="wvh")
            nc.scalar.dma_start(wvh[:], wvh_d[:])
            wvl = wp.tile([128, 4, 2, FPC], fp8, tag="wvl")
            nc.scalar.dma_start(wvl[:], wvl_d[:])
            w2 = wp.tile([128, 4, 8, 128], bf16, tag="w2")
            nc.scalar.dma_start(w2[:], w2_d[:])
            maskT = wp.tile([C, C], f32, tag="maskT")
            nc.gpsimd.dma_start(maskT[:], maskT_d[:])
            id128 = wp.tile([128, 128], bf16, tag="id128")
            nc.gpsimd.dma_start(id128[:], id128_d[:])
            wb8 = wp.tile([128, 2, 128], fp8, tag="wb8")
            nc.vector.memset(wb8[:], 0.0)
            nc.vector.memset(wb8[0:1, 0, :], 32.0)
            xb8 = wp.tile([128, 2, ST], fp8, tag="xb8")
            nc.vector.memset(xb8[:], 0.0)
            nc.vector.memset(xb8[0:1, 0, :], 1.0)
            ones32 = wp.tile([128, 1], bf16, tag="ones32")
            nc.vector.memset(ones32[:], 1.0 / 32.0)
            bm1 = wp.tile([128, 1], f32, tag="bm1")
            nc.vector.memset(bm1[:], float(np.log(32.0) - 1.0))

            # persistent scan state [S | z/32] per head pair, in PSUM.
            # Full-bank tile (2048B row pitch) so pending-zero bookkeeping
            # stays row-aligned for base_partition=64 matmul outputs.
            Sz_t = szp.tile([128, 512], f32, tag="Sz")
            Sz = Sz_t[:, 0:260].rearrange("p (j e) -> p j e", e=DH + 1)
            sz8_init = szsp.tile([128, 4, DH + 1], bf16, tag="Sz8", name="sz8i")
            nc.vector.memset(sz8_init[:], 0.0)
            szh = [sz8_init]  # rolling snapshot holder

            def emit_proj_tiles(st):
                t0 = st * ST
                xhi = xp.tile([128, 4, 2, ST], fp8, tag="xhi", name=f"xhi{st}")
                nc.sync.dma_start(xhi[:], xhi_d[:, :, :, t0 : t0 + ST])
                xlo = xp.tile([128, 4, 2, ST], fp8, tag="xlo", name=f"xlo{st}")
                nc.sync.dma_start(xlo[:], xlo_d[:, :, :, t0 : t0 + ST])
                qTb = qp.tile([128, 4, ST], bf16, tag="qTb", name=f"qTb{st}")
                kTb = qp.tile([128, 4, ST], bf16, tag="kTb", name=f"kTb{st}")
                v1 = vp.tile([128, NCH, 8, DH], bf16, tag="v1", name=f"v1{st}")
                return dict(xhi=xhi, xlo=xlo, qTb=qTb, kTb=kTb, v1=v1)

            def emit_fc(T, fc):
                pq = projp.tile([128, ST], f32, tag="proj", name=f"pq_{fc}")
                for hh in range(2):
                    sl = slice(256 * hh, 256 * hh + 256)
                    for j in range(4):
                        nc.tensor.matmul(
                            pq[:, sl],
                            wqk[:, fc, j, :, :],
                            T["xhi"][:, j, :, sl],
                            start=(hh == 0 and j == 0),
                            stop=False,
                            perf_mode=PM.DoubleRow,
                            skip_group_check=True,
                        )
                    nc.tensor.matmul(
                        pq[:, sl],
                        wb8[:],
                        xb8[:, :, sl],
                        start=False,
                        stop=(hh == 1),
                        perf_mode=PM.DoubleRow,
                        skip_group_check=True,
                    )
                texp = scr.tile([128, ST], f32, tag="texp", name=f"texp{fc}")
                nc.scalar.activation(
                    texp[:], pq[:], Act.Exp, bias=bm1[:], scale=1.0 / 32.0
                )
                dst = T["qTb"][:, fc, :] if fc < 4 else T["kTb"][:, fc - 4, :]
                nc.vector.scalar_tensor_tensor(
                    dst, texp[:], 32.0, pq[:], Alu.min, Alu.max
                )

            def emit_v(T, tcc):
                pv = projp.tile([128, FPC], f32, tag="proj", name=f"pv{tcc}")
                xc_hi = T["xhi"][:, :, :, tcc * 128 : (tcc + 1) * 128]
                xc_lo = T["xlo"][:, :, :, tcc * 128 : (tcc + 1) * 128]
                n = 0
                for hh in range(2):
                    sl = slice(256 * hh, 256 * hh + 256)
                    for xa, wv in ((xc_hi, wvh), (xc_hi, wvl), (xc_lo, wvh)):
                        for j in range(4):
                            n += 1
                            nc.tensor.matmul(
                                pv[:, sl],
                                xa[:, j, :, :],
                                wv[:, j, :, sl],
                                start=(n == 1),
                                stop=(n == 24),
                                perf_mode=PM.DoubleRow,
                                skip_group_check=True,
                            )
                nc.scalar.mul(
                    T["v1"][:, tcc, :, :],
                    pv[:].rearrange("p (h e) -> p h e", e=DH),
                    1.0 / 128.0,
                )

            def emit_chunk(T, st, tcc, outT_sb):
                cg = st * NCH + tcc
                tsl = slice(tcc * 128, (tcc + 1) * 128)
                qTb, kTb, v1 = T["qTb"], T["kTb"], T["v1"]
                sz8_prev = szh[0]
                misc = miscp.tile([128, 264], f32, tag="misc", name=f"misc{cg}")
                ktr = misc[:, 0:256].bitcast(bf16).rearrange(
                    "p (j t) -> p j t", j=4
                )  # [128, 4, 128] bf16
                den = misc[:, 256:264]  # [128, 8] f32
                trp_t = trpp.tile([128, 256], f32, tag="trp", name=f"trp{cg}")
                trp = trp_t[:].bitcast(bf16).rearrange(
                    "p (g t) -> p g t", g=4
                )  # [128, 4, 128] bf16

                # -- k transpose to token-major --
                for j in range(4):
                    nc.tensor.matmul(
                        ktr[:, j, :],
                        kTb[:, j, tsl],
                        id128[:],
                        is_transpose=True,
                        start=(j == 0),
                        stop=(j == 3),
                        skip_group_check=True,
                    )
                ksb = kp.tile([128, 8, DH], bf16, tag="ksb", name=f"ksb{cg}")
                nc.vector.tensor_copy(
                    ksb[:], ktr[:].rearrange("p j (hh e) -> p (j hh) e", e=DH)
                )

                # -- A^T = K^T Q; bank g = heads of parity g (a PE tile-row
                # config may not change within a PSUM bank) --
                ATp = [
                    atp.tile([128, 4, C], f32, tag="ATp", name=f"atp{cg}_{g}")
                    for g in range(2)
                ]
                for h in range(8):
                    po = 64 * (h % 2)
                    fq = h // 2
                    nc.tensor.matmul(
                        ATp[h % 2][:, h // 2, :],
                        kTb[po : po + 64, fq, tsl],
                        qTb[po : po + 64, fq, tsl],
                        start=(h // 2 == 0),
                        stop=(h // 2 == 3),
                        skip_group_check=True,
                    )
                ATs = atsp.tile([128, 8, C], bf16, tag="ATs", name=f"ats{cg}")
                for g in range(2):
                    nc.vector.tensor_tensor(
                        ATs[:, 4 * g : 4 * g + 4, :],
                        ATp[g][:],
                        maskT[:].unsqueeze(1).broadcast_to([C, 4, C]),
                        Alu.mult,
                    )

                # -- O = A_m V + Q S8 (one bank, 8 heads) --
                Ob = obp.tile([128, 8, DH], f32, tag="Ob", name=f"ob{cg}")
                for h in range(8):
                    po = 64 * (h % 2)
                    fq = h // 2
                    nc.tensor.matmul(
                        Ob[:, h, :],
                        ATs[:, 4 * (h % 2) + h // 2, :],
                        v1[:, tcc, h, :],
                        start=(h == 0),
                        stop=False,
                        skip_group_check=True,
                    )
                    nc.tensor.matmul(
                        Ob[:, h, :],
                        qTb[po : po + 64, fq, tsl],
                        sz8_prev[po : po + 64, h // 2, 0:DH],
                        start=False,
                        stop=(h == 7),
                        skip_group_check=True,
                    )

                # -- den = A_m^T ones/32 + q . z8 --
                for h in range(8):
                    po = 64 * (h % 2)
                    fq = h // 2
                    nc.tensor.matmul(
                        den[:, h : h + 1],
                        ATs[:, 4 * (h % 2) + h // 2, :],
                        ones32[:],
                        start=(h == 0),
                        stop=False,
                        skip_group_check=True,
                    )
                    nc.tensor.matmul(
                        den[:, h : h + 1],
                        qTb[po : po + 64, fq, tsl],
                        sz8_prev[po : po + 64, h // 2, DH : DH + 1],
                        start=False,
                        stop=(h == 7),
                        skip_group_check=True,
                    )
                dens = dp.tile([C, 8], f32, tag="dens", name=f"dens{cg}")
                nc.vector.tensor_scalar(
                    dens[:], den[:], EPS / 32.0, 8.0, Alu.max, Alu.mult
                )
                rden = dp.tile([C, 8], f32, tag="rden", name=f"rden{cg}")
                nc.vector.reciprocal(rden[:], dens[:])

                # -- state update into the persistent Sz bank --
                for h in range(8):
                    po = 64 * (h % 2)
                    j = h // 2
                    nc.tensor.matmul(
                        Sz[po : po + 64, j, 0:DH],
                        ksb[:, h, :],
                        v1[:, tcc, h, :],
                        start=(cg == 0 and h <= 1),
                        stop=False,
                        skip_group_check=True,
                    )
                    nc.tensor.matmul(
                        Sz[po : po + 64, j, DH : DH + 1],
                        ksb[:, h, :],
                        ones32[:],
                        start=False,
                        stop=(cg == NST * NCH - 1 and h == 7),
                        skip_group_check=True,
                    )
                sz8_new = szsp.tile(
                    [128, 4, DH + 1], bf16, tag="Sz8", name=f"sz8_{cg}"
                )
                nc.scalar.copy(sz8_new[:], Sz[:])
                szh[0] = sz8_new

                # -- normalize -> bf16 o_c --
                oc8 = ocp.tile([C, 8, DH], bf16, tag="oc8", name=f"oc8{cg}")
                nc.vector.tensor_tensor(
                    oc8[:],
                    Ob[:],
                    rden[:].unsqueeze(2).broadcast_to([C, 8, DH]),
                    Alu.mult,
                )

                # -- transpose o_c to feature-major --
                oc_flat = oc8[:].rearrange("p h e -> p (h e)")
                for g in range(4):
                    nc.tensor.matmul(
                        trp[:, g, :],
                        oc_flat[:, g * 128 : (g + 1) * 128],
                        id128[:],
                        is_transpose=True,
                        start=(g == 0),
                        stop=(g == 3),
                        skip_group_check=True,
                    )
                nc.vector.tensor_copy(outT_sb[:, :, tsl], trp[:])

            def emit_outproj(st, outT_sb):
                t0 = st * ST
                o3sb = o3p.tile([128, 8, ST], bf16, tag="o3sb", name=f"o3sb{st}")
                for oc in range(8):
                    po3 = projp.tile([128, ST], f32, tag="proj", name=f"po3_{oc}")
                    for j in range(4):
                        nc.tensor.matmul(
                            po3[:],
                            w2[:, j, oc, :],
                            outT_sb[:, j, :],
                            start=(j == 0),
                            stop=(j == 3),
                        )
                    nc.scalar.copy(o3sb[:, oc, :], po3[:])
                nc.sync.dma_start(
                    outT_d[:, t0 : t0 + ST].rearrange("(c p) t -> p c t", p=128),
                    o3sb[:],
                )

            # ---- software-pipelined emission ----
            tiles = {0: emit_proj_tiles(0)}
            for fc in range(8):
                emit_fc(tiles[0], fc)
            for tcc in range(NCH):
                emit_v(tiles[0], tcc)
            for st in range(NST):
                T = tiles.pop(st)
                Tn = None
                if st + 1 < NST:
                    Tn = emit_proj_tiles(st + 1)
                    tiles[st + 1] = Tn
                outT_sb = osp.tile(
                    [128, 4, ST], bf16, tag="outT_sb", name=f"osb{st}"
                )
                for tcc in range(NCH):
                    emit_chunk(T, st, tcc, outT_sb)
                    if Tn is not None:
                        emit_fc(Tn, 2 * tcc)
                        emit_fc(Tn, 2 * tcc + 1)
                        if tcc % 2 == 1:
                            emit_v(Tn, tcc - 1)
                            emit_v(Tn, tcc)
                emit_outproj(st, outT_sb)

    nc.finalize()
    return nc


def _get_nc():
    if "nc" not in _CACHE:
        _CACHE["nc"] = _build_nc()
    return _CACHE["nc"]


def _pack_x(xT):
    """xT: [1024, 4096] f32 -> hi/lo fp8 in [128, 4, 2, L] layout."""
    xr = xT.reshape(4, 2, 128, L).transpose(2, 0, 1, 3)  # [128, 4, 2, L]
    hi = xr.astype(F8)
    lo = (xr - hi.astype(np.float32)).astype(F8)
    return np.ascontiguousarray(hi), np.ascontiguousarray(lo)


def _make_in_maps(x, w_qkv, w_out):
    maskT = np.triu(np.ones((C, C), np.float32))  # maskT[j,i] = 1 if j <= i
    id128 = np.eye(128, dtype=np.float32)
    in_maps = []
    for core in range(8):
        b, g = core // 2, core % 2
        xT = np.ascontiguousarray(x[b].T).astype(np.float32)
        xhi, xlo = _pack_x(xT)
        # q/k weights x32 (fp8 subnormal avoidance), transposed
        W1 = np.concatenate(
            [
                w_qkv[512 * g : 512 * (g + 1)],
                w_qkv[1024 + 512 * g : 1024 + 512 * (g + 1)],
            ],
            axis=0,
        ).T.astype(np.float32)  # [1024 d, 1024 f]
        wqk = (32.0 * W1).reshape(4, 2, 128, 8, 128).transpose(2, 3, 0, 1, 4)
        wqk8 = np.ascontiguousarray(wqk).astype(F8)
        # v weights x32, hi/lo split
        Wv = (32.0 * w_qkv[2048 + 512 * g : 2048 + 512 * (g + 1)].T).astype(
            np.float32
        )  # [1024 d, 512 f]
        Wvr = Wv.reshape(4, 2, 128, FPC).transpose(2, 0, 1, 3)  # [128,4,2,512]
        wvh = Wvr.astype(F8)
        wvl = (Wvr - wvh.astype(np.float32)).astype(F8)
        # output projection weights [512 f, 1024 oc] -> [128, 4, 8, 128]
        W2 = w_out[:, 512 * g : 512 * (g + 1)].T.astype(np.float32)
        w2r = W2.reshape(4, 128, 8, 128).transpose(1, 0, 2, 3)
        in_maps.append(
            {
                "xhi": xhi,
                "xlo": np.ascontiguousarray(xlo),
                "wqk": wqk8,
                "wvh": np.ascontiguousarray(wvh),
                "wvl": np.ascontiguousarray(wvl),
                "w2": np.ascontiguousarray(w2r).astype(BFNP),
                "maskT": maskT,
                "id128": id128.astype(BFNP),
            }
        )
    return in_maps


def _run(inputs, trace=False):
    from concourse.bass_utils import run_bass_kernel_spmd

    nc = _get_nc()
    in_maps = _make_in_maps(inputs["x"], inputs["w_qkv"], inputs["w_out"])
    res = run_bass_kernel_spmd(nc, in_maps, core_ids=list(range(8)), trace=trace)
    out = np.empty((B, L, D), np.float32)
    for b in range(B):
        p0 = res.results[2 * b]["outT"].astype(np.float32)
        p1 = res.results[2 * b + 1]["outT"].astype(np.float32)
        out[b] = (p0 + p1).T
    return out, res


def kernel(x, w_qkv, w_out):
    out, _ = _run({"x": x, "w_qkv": w_qkv, "w_out": w_out})
    return out


# revision 15
# speedup vs baseline: 1.8065x; 1.0534x over previous
"""TRN2 Bass kernel for nn_LinearAttention (B=4, L=4096, D=1024, H=16, dh=64).

Sharding: core c = batch c//2, heads (c%2)*8..+8. Zero cross-core comms; the
two half-head partial output projections per batch are summed on the host.

Per-core pipeline (super-tiles of 512 tokens, scan chunks of 128):
  1. QKV projection:
     - q/k: fp8e4 DoubleRow matmuls (0.5 cyc/row, K=256 per call). Weights are
       scaled x32 on the host (plain w_qkv values sit in e4m3's subnormal
       range and quantize at 6-50% error; x32 moves them to normals). A bias
       ones-row (weight 32) makes PSUM hold 32(z+1); phi is exact in 2 passes
       via 32*phi(z) = max(min(32*e^z, 32), 32z+32):
       Act Exp(scale=1/32, bias=ln32-1) then one DVE scalar_tensor_tensor.
       The x32 on q/k cancels between numerator and denominator.
     - v: 3-term compensated fp8 DoubleRow (x_hi w_hi + x_hi w_lo + x_lo w_hi)
       giving ~bf16 accuracy at 0.75x bf16 matmul cost; the copy to bf16
       applies 1/128 (un-scales the x32 and pre-divides v by 4 for range).
  2. Chunked linear attention, bf16 operands / f32 PSUM:
     A^T = K^T Q per head (heads grouped by partition parity per PSUM bank --
     a PE tile-row config may not change within a bank) -> masked-copy (DVE,
     doubles as the PSUM->SBUF move) -> O = A_m V + Q S8, all 8 heads in one
     bank; den via tiny matmuls (masked-A^T . ones/32 + q . z8); state
     [S | z/32] lives PERSISTENTLY in one PSUM bank (start only at chunk 0)
     with a per-chunk bf16 snapshot for the next chunk's Q S / q.z reads.
     den' = max(den, eps/32)*8 restores every scale factor exactly.
  3. PE-transpose o (bf16) -> output projection in bf16 -> bf16 partials
     DMA'd out; the host upcasts and sums core pairs in f32.

The emission order is software-pipelined: attention chunks of super-tile s
are interleaved with the projection of super-tile s+1 so the PE never waits
on the phi-combine consumer chain (engines execute their streams in order).
"""
import sys

sys.path.insert(0, "/opt/trn_rl_repo")
import numpy as np
import ml_dtypes

F8 = ml_dtypes.float8_e4m3
BFNP = ml_dtypes.bfloat16

D = 1024
L = 4096
B = 4
H = 16
DH = 64
FPC = 512          # features per core (8 heads x 64)
C = 128            # scan chunk
ST = 512           # super-tile tokens
NCH = ST // C      # 4
NST = L // ST      # 8
EPS = 1e-6

_CACHE = {}


def _build_nc():
    import concourse.bacc as bacc
    import concourse.mybir as mybir
    import concourse.tile as tile

    dt = mybir.dt
    f32, fp8, bf16 = dt.float32, dt.float8e4, dt.bfloat16
    Alu = mybir.AluOpType
    Act = mybir.ActivationFunctionType
    PM = mybir.MatmulPerfMode

    nc = bacc.Bacc("TRN2", target_bir_lowering=False, debug=True)

    xhi_d = nc.dram_tensor("xhi", [128, 4, 2, L], fp8, kind="ExternalInput")
    xlo_d = nc.dram_tensor("xlo", [128, 4, 2, L], fp8, kind="ExternalInput")
    wqk_d = nc.dram_tensor("wqk", [128, 8, 4, 2, 128], fp8, kind="ExternalInput")
    wvh_d = nc.dram_tensor("wvh", [128, 4, 2, FPC], fp8, kind="ExternalInput")
    wvl_d = nc.dram_tensor("wvl", [128, 4, 2, FPC], fp8, kind="ExternalInput")
    w2_d = nc.dram_tensor("w2", [128, 4, 8, 128], bf16, kind="ExternalInput")
    maskT_d = nc.dram_tensor("maskT", [C, C], f32, kind="ExternalInput")
    id128_d = nc.dram_tensor("id128", [128, 128], bf16, kind="ExternalInput")
    outT_d = nc.dram_tensor("outT", [D, L], bf16, kind="ExternalOutput")

    from contextlib import ExitStack

    with tile.TileContext(nc) as tc, ExitStack() as es:
        if True:
            wp = es.enter_context(tc.tile_pool(name="wp", bufs=1))
            xp = es.enter_context(tc.tile_pool(name="xp", bufs=2))
            qp = es.enter_context(tc.tile_pool(name="qp", bufs=4))
            scr = es.enter_context(tc.tile_pool(name="scr", bufs=3))
            vp = es.enter_context(tc.tile_pool(name="vp", bufs=2))
            kp = es.enter_context(tc.tile_pool(name="kp", bufs=3))
            atsp = es.enter_context(tc.tile_pool(name="atsp", bufs=3))
            szsp = es.enter_context(tc.tile_pool(name="szsp", bufs=3))
            dp = es.enter_context(tc.tile_pool(name="dp", bufs=4))
            ocp = es.enter_context(tc.tile_pool(name="ocp", bufs=3))
            osp = es.enter_context(tc.tile_pool(name="osp", bufs=2))
            o3p = es.enter_context(tc.tile_pool(name="o3p", bufs=2))
            projp = es.enter_context(tc.tile_pool(name="projp", bufs=2, space="PSUM"))
            atp = es.enter_context(tc.tile_pool(name="atp", bufs=2, space="PSUM"))
            obp = es.enter_context(tc.tile_pool(name="obp", bufs=1, space="PSUM"))
            szp = es.enter_context(tc.tile_pool(name="szp", bufs=1, space="PSUM"))
            miscp = es.enter_context(tc.tile_pool(name="miscp", bufs=1, space="PSUM"))
            trpp = es.enter_context(tc.tile_pool(name="trpp", bufs=1, space="PSUM"))

            # ---- resident constants / weights ----
            wqk = wp.tile([128, 8, 4, 2, 128], fp8, tag="wqk")
            nc.gpsimd.dma_start(wqk[:], wqk_d[:])
            wvh = wp.tile([128, 4, 2, FPC], fp8, tag="wvh")
            nc.scalar.dma_start(wvh[:], wvh_d[:])
            wvl = wp.tile([128, 4, 2, FPC], fp8, tag="wvl")
            nc.scalar.dma_start(wvl[:], wvl_d[:])
            w2 = wp.tile([128, 4, 8, 128], bf16, tag="w2")
            nc.scalar.dma_start(w2[:], w2_d[:])
            maskT = wp.tile([C, C], f32, tag="maskT")
            nc.gpsimd.dma_start(maskT[:], maskT_d[:])
            id128 = wp.tile([128, 128], bf16, tag="id128")
            nc.gpsimd.dma_start(id128[:], id128_d[:])
            wb8 = wp.tile([128, 2, 128], fp8, tag="wb8")
            nc.vector.memset(wb8[:], 0.0)
            nc.vector.memset(wb8[0:1, 0, :], 32.0)
            xb8 = wp.tile([128, 2, ST], fp8, tag="xb8")
            nc.vector.memset(xb8[:], 0.0)
            nc.vector.memset(xb8[0:1, 0, :], 1.0)
            ones32 = wp.tile([128, 1], bf16, tag="ones32")
            nc.vector.memset(ones32[:], 1.0 / 32.0)
            bm1 = wp.tile([128, 1], f32, tag="bm1")
            nc.vector.memset(bm1[:], float(np.log(32.0) - 1.0))

            # persistent scan state [S | z/32] per head pair, in PSUM.
            # Full-bank tile (2048B row pitch) so pending-zero bookkeeping
            # stays row-aligned for base_partition=64 matmul outputs.
            Sz_t = szp.tile([128, 512], f32, tag="Sz")
            Sz = Sz_t[:, 0:260].rearrange("p (j e) -> p j e", e=DH + 1)
            sz8_init = szsp.tile([128, 4, DH + 1], bf16, tag="Sz8", name="sz8i")
            nc.vector.memset(sz8_init[:], 0.0)
            szh = [sz8_init]  # rolling snapshot holder

            def emit_proj_tiles(st):
                t0 = st * ST
                xhi = xp.tile([128, 4, 2, ST], fp8, tag="xhi", name=f"xhi{st}")
                nc.sync.dma_start(xhi[:], xhi_d[:, :, :, t0 : t0 + ST])
                xlo = xp.tile([128, 4, 2, ST], fp8, tag="xlo", name=f"xlo{st}")
                nc.sync.dma_start(xlo[:], xlo_d[:, :, :, t0 : t0 + ST])
                qTb = qp.tile([128, 4, ST], bf16, tag="qTb", name=f"qTb{st}")
                kTb = qp.tile([128, 4, ST], bf16, tag="kTb", name=f"kTb{st}")
                v1 = vp.tile([128, NCH, 8, DH], bf16, tag="v1", name=f"v1{st}")
                return dict(xhi=xhi, xlo=xlo, qTb=qTb, kTb=kTb, v1=v1)

            def emit_fc(T, fc):
                pq = projp.tile([128, ST], f32, tag="proj", name=f"pq_{fc}")
                for hh in range(2):
                    sl = slice(256 * hh, 256 * hh + 256)
                    for j in range(4):
                        nc.tensor.matmul(
                            pq[:, sl],
                            wqk[:, fc, j, :, :],
                            T["xhi"][:, j, :, sl],
                            start=(hh == 0 and j == 0),
                            stop=False,
                            perf_mode=PM.DoubleRow,
                            skip_group_check=True,
                        )
                    nc.tensor.matmul(
                        pq[:, sl],
                        wb8[:],
                        xb8[:, :, sl],
                        start=False,
                        stop=(hh == 1),
                        perf_mode=PM.DoubleRow,
                        skip_group_check=True,
                    )
                texp = scr.tile([128, ST], f32, tag="texp", name=f"texp{fc}")
                nc.scalar.activation(
                    texp[:], pq[:], Act.Exp, bias=bm1[:], scale=1.0 / 32.0
                )
                dst = T["qTb"][:, fc, :] if fc < 4 else T["kTb"][:, fc - 4, :]
                nc.vector.scalar_tensor_tensor(
                    dst, texp[:], 32.0, pq[:], Alu.min, Alu.max
                )

            def emit_v(T, tcc):
                pv = projp.tile([128, FPC], f32, tag="proj", name=f"pv{tcc}")
                xc_hi = T["xhi"][:, :, :, tcc * 128 : (tcc + 1) * 128]
                xc_lo = T["xlo"][:, :, :, tcc * 128 : (tcc + 1) * 128]
                n = 0
                for hh in range(2):
                    sl = slice(256 * hh, 256 * hh + 256)
                    for xa, wv in ((xc_hi, wvh), (xc_hi, wvl), (xc_lo, wvh)):
                        for j in range(4):
                            n += 1
                            nc.tensor.matmul(
                                pv[:, sl],
                                xa[:, j, :, :],
                                wv[:, j, :, sl],
                                start=(n == 1),
                                stop=(n == 24),
                                perf_mode=PM.DoubleRow,
                                skip_group_check=True,
                            )
                nc.scalar.mul(
                    T["v1"][:, tcc, :, :],
                    pv[:].rearrange("p (h e) -> p h e", e=DH),
                    1.0 / 128.0,
                )

            def emit_chunk(T, st, tcc, outT_sb):
                cg = st * NCH + tcc
                tsl = slice(tcc * 128, (tcc + 1) * 128)
                qTb, kTb, v1 = T["qTb"], T["kTb"], T["v1"]
                sz8_prev = szh[0]
                misc = miscp.tile([128, 264], f32, tag="misc", name=f"misc{cg}")
                ktr = misc[:, 0:256].bitcast(bf16).rearrange(
                    "p (j t) -> p j t", j=4
                )  # [128, 4, 128] bf16
                den = misc[:, 256:264]  # [128, 8] f32
                trp_t = trpp.tile([128, 256], f32, tag="trp", name=f"trp{cg}")
                trp = trp_t[:].bitcast(bf16).rearrange(
                    "p (g t) -> p g t", g=4
                )  # [128, 4, 128] bf16

                # -- k transpose to token-major --
                for j in range(4):
                    nc.tensor.matmul(
                        ktr[:, j, :],
                        kTb[:, j, tsl],
                        id128[:],
                        is_transpose=True,
                        start=(j == 0),
                        stop=(j == 3),
                        skip_group_check=True,
                    )
                ksb = kp.tile([128, 8, DH], bf16, tag="ksb", name=f"ksb{cg}")
                nc.scalar.copy(
                    ksb[:], ktr[:].rearrange("p j (hh e) -> p (j hh) e", e=DH)
                )

                # -- A^T = K^T Q; bank g = heads of parity g (a PE tile-row
                # config may not change within a PSUM bank) --
                ATp = [
                    atp.tile([128, 4, C], f32, tag="ATp", name=f"atp{cg}_{g}")
                    for g in range(2)
                ]
                for h in range(8):
                    po = 64 * (h % 2)
                    fq = h // 2
                    nc.tensor.matmul(
                        ATp[h % 2][:, h // 2, :],
                        kTb[po : po + 64, fq, tsl],
                        qTb[po : po + 64, fq, tsl],
                        start=(h // 2 == 0),
                        stop=(h // 2 == 3),
                        skip_group_check=True,
                    )
                ATs = atsp.tile([128, 8, C], bf16, tag="ATs", name=f"ats{cg}")
                for g in range(2):
                    nc.vector.tensor_tensor(
                        ATs[:, 4 * g : 4 * g + 4, :],
                        ATp[g][:],
                        maskT[:].unsqueeze(1).broadcast_to([C, 4, C]),
                        Alu.mult,
                    )

                # -- O = A_m V + Q S8 (one bank, 8 heads) --
                Ob = obp.tile([128, 8, DH], f32, tag="Ob", name=f"ob{cg}")
                for h in range(8):
                    po = 64 * (h % 2)
                    fq = h // 2
                    nc.tensor.matmul(
                        Ob[:, h, :],
                        ATs[:, 4 * (h % 2) + h // 2, :],
                        v1[:, tcc, h, :],
                        start=(h == 0),
                        stop=False,
                        skip_group_check=True,
                    )
                    nc.tensor.matmul(
                        Ob[:, h, :],
                        qTb[po : po + 64, fq, tsl],
                        sz8_prev[po : po + 64, h // 2, 0:DH],
                        start=False,
                        stop=(h == 7),
                        skip_group_check=True,
                    )

                # -- den = A_m^T ones/32 + q . z8 --
                for h in range(8):
                    po = 64 * (h % 2)
                    fq = h // 2
                    nc.tensor.matmul(
                        den[:, h : h + 1],
                        ATs[:, 4 * (h % 2) + h // 2, :],
                        ones32[:],
                        start=(h == 0),
                        stop=False,
                        skip_group_check=True,
                    )
                    nc.tensor.matmul(
                        den[:, h : h + 1],
                        qTb[po : po + 64, fq, tsl],
                        sz8_prev[po : po + 64, h // 2, DH : DH + 1],
                        start=False,
                        stop=(h == 7),
                        skip_group_check=True,
                    )
                dens = dp.tile([C, 8], f32, tag="dens", name=f"dens{cg}")
                nc.vector.tensor_scalar(
                    dens[:], den[:], EPS / 32.0, 8.0, Alu.max, Alu.mult
                )
                rden = dp.tile([C, 8], f32, tag="rden", name=f"rden{cg}")
                nc.vector.reciprocal(rden[:], dens[:])

                # -- state update into the persistent Sz bank --
                for h in range(8):
                    po = 64 * (h % 2)
                    j = h // 2
                    nc.tensor.matmul(
                        Sz[po : po + 64, j, 0:DH],
                        ksb[:, h, :],
                        v1[:, tcc, h, :],
                        start=(cg == 0 and h <= 1),
                        stop=False,
                        skip_group_check=True,
                    )
                    nc.tensor.matmul(
                        Sz[po : po + 64, j, DH : DH + 1],
                        ksb[:, h, :],
                        ones32[:],
                        start=False,
                        stop=(cg == NST * NCH - 1 and h == 7),
                        skip_group_check=True,
                    )
                sz8_new = szsp.tile(
                    [128, 4, DH + 1], bf16, tag="Sz8", name=f"sz8_{cg}"
                )
                nc.scalar.copy(sz8_new[:], Sz[:])
                szh[0] = sz8_new

                # -- normalize -> bf16 o_c --
                oc8 = ocp.tile([C, 8, DH], bf16, tag="oc8", name=f"oc8{cg}")
                nc.vector.tensor_tensor(
                    oc8[:],
                    Ob[:],
                    rden[:].unsqueeze(2).broadcast_to([C, 8, DH]),
                    Alu.mult,
                )

                # -- transpose o_c to feature-major --
                oc_flat = oc8[:].rearrange("p h e -> p (h e)")
                for g in range(4):
                    nc.tensor.matmul(
                        trp[:, g, :],
                        oc_flat[:, g * 128 : (g + 1) * 128],
                        id128[:],
                        is_transpose=True,
                        start=(g == 0),
                        stop=(g == 3),
                        skip_group_check=True,
                    )
                nc.scalar.copy(outT_sb[:, :, tsl], trp[:])

            def emit_outproj(st, outT_sb):
                t0 = st * ST
                o3sb = o3p.tile([128, 8, ST], bf16, tag="o3sb", name=f"o3sb{st}")
                for oc in range(8):
                    po3 = projp.tile([128, ST], f32, tag="proj", name=f"po3_{oc}")
                    for j in range(4):
                        nc.tensor.matmul(
                            po3[:],
                            w2[:, j, oc, :],
                            outT_sb[:, j, :],
                            start=(j == 0),
                            stop=(j == 3),
                        )
                    nc.scalar.copy(o3sb[:, oc, :], po3[:])
                nc.sync.dma_start(
                    outT_d[:, t0 : t0 + ST].rearrange("(c p) t -> p c t", p=128),
                    o3sb[:],
                )

            # ---- software-pipelined emission ----
            tiles = {0: emit_proj_tiles(0)}
            for fc in range(8):
                emit_fc(tiles[0], fc)
            for tcc in range(NCH):
                emit_v(tiles[0], tcc)
            for st in range(NST):
                T = tiles.pop(st)
                Tn = None
                if st + 1 < NST:
                    Tn = emit_proj_tiles(st + 1)
                    tiles[st + 1] = Tn
                outT_sb = osp.tile(
                    [128, 4, ST], bf16, tag="outT_sb", name=f"osb{st}"
                )
                for tcc in range(NCH):
                    emit_chunk(T, st, tcc, outT_sb)
                    if Tn is not None:
                        emit_fc(Tn, 2 * tcc)
                        emit_fc(Tn, 2 * tcc + 1)
                        if tcc % 2 == 1:
                            emit_v(Tn, tcc - 1)
                            emit_v(Tn, tcc)
                emit_outproj(st, outT_sb)

    nc.finalize()
    return nc


def _get_nc():
    if "nc" not in _CACHE:
        _CACHE["nc"] = _build_nc()
    return _CACHE["nc"]


def _pack_x(xT):
    """xT: [1024, 4096] f32 -> hi/lo fp8 in [128, 4, 2, L] layout."""
    xr = xT.reshape(4, 2, 128, L).transpose(2, 0, 1, 3)  # [128, 4, 2, L]
    hi = xr.astype(F8)
    lo = (xr - hi.astype(np.float32)).astype(F8)
    return np.ascontiguousarray(hi), np.ascontiguousarray(lo)


def _make_in_maps(x, w_qkv, w_out):
    maskT = np.triu(np.ones((C, C), np.float32))  # maskT[j,i] = 1 if j <= i
    id128 = np.eye(128, dtype=np.float32)
    in_maps = []
    for core in range(8):
        b, g = core // 2, core % 2
        xT = np.ascontiguousarray(x[b].T).astype(np.float32)
        xhi, xlo = _pack_x(xT)
        # q/k weights x32 (fp8 subnormal avoidance), transposed
        W1 = np.concatenate(
            [
                w_qkv[512 * g : 512 * (g + 1)],
                w_qkv[1024 + 512 * g : 1024 + 512 * (g + 1)],
            ],
            axis=0,
        ).T.astype(np.float32)  # [1024 d, 1024 f]
        wqk = (32.0 * W1).reshape(4, 2, 128, 8, 128).transpose(2, 3, 0, 1, 4)
        wqk8 = np.ascontiguousarray(wqk).astype(F8)
        # v weights x32, hi/lo split
        Wv = (32.0 * w_qkv[2048 + 512 * g : 2048 + 512 * (g + 1)].T).astype(
            np.float32
        )  # [1024 d, 512 f]
        Wvr = Wv.reshape(4, 2, 128, FPC).transpose(2, 0, 1, 3)  # [128,4,2,512]
        wvh = Wvr.astype(F8)
        wvl = (Wvr - wvh.astype(np.float32)).astype(F8)
        # output projection weights [512 f, 1024 oc] -> [128, 4, 8, 128]
        W2 = w_out[:, 512 * g : 512 * (g + 1)].T.astype(np.float32)
        w2r = W2.reshape(4, 128, 8, 128).transpose(1, 0, 2, 3)
        in_maps.append(
            {
                "xhi": xhi,
                "xlo": np.ascontiguousarray(xlo),
                "wqk": wqk8,
                "wvh": np.ascontiguousarray(wvh),
                "wvl": np.ascontiguousarray(wvl),
                "w2": np.ascontiguousarray(w2r).astype(BFNP),
                "maskT": maskT,
                "id128": id128.astype(BFNP),
            }
        )
    return in_maps


def _run(inputs, trace=False):
    from concourse.bass_utils import run_bass_kernel_spmd

    nc = _get_nc()
    in_maps = _make_in_maps(inputs["x"], inputs["w_qkv"], inputs["w_out"])
    res = run_bass_kernel_spmd(nc, in_maps, core_ids=list(range(8)), trace=trace)
    out = np.empty((B, L, D), np.float32)
    for b in range(B):
        p0 = res.results[2 * b]["outT"].astype(np.float32)
        p1 = res.results[2 * b + 1]["outT"].astype(np.float32)
        out[b] = (p0 + p1).T
    return out, res


def kernel(x, w_qkv, w_out):
    out, _ = _run({"x": x, "w_qkv": w_qkv, "w_out": w_out})
    return out


# revision 16
# speedup vs baseline: 1.8887x; 1.0455x over previous
"""TRN2 Bass kernel for nn_LinearAttention (B=4, L=4096, D=1024, H=16, dh=64).

Sharding: core c = batch c//2, heads (c%2)*8..+8. Zero cross-core comms; the
two half-head partial output projections per batch are summed on the host.

Per-core pipeline (super-tiles of 512 tokens, scan chunks of 128):
  1. QKV projection:
     - q/k: fp8e4 DoubleRow matmuls (0.5 cyc/row, K=256 per call). Weights are
       scaled x32 on the host (plain w_qkv values sit in e4m3's subnormal
       range and quantize at 6-50% error; x32 moves them to normals). A bias
       ones-row (weight 32) makes PSUM hold 32(z+1); phi is exact in 2 passes
       via 32*phi(z) = max(min(32*e^z, 32), 32z+32):
       Act Exp(scale=1/32, bias=ln32-1) then one DVE scalar_tensor_tensor.
       The x32 on q/k cancels between numerator and denominator.
     - v: 3-term compensated fp8 DoubleRow (x_hi w_hi + x_hi w_lo + x_lo w_hi)
       giving ~bf16 accuracy at 0.75x bf16 matmul cost; the copy to bf16
       applies 1/128 (un-scales the x32 and pre-divides v by 4 for range).
  2. Chunked linear attention, bf16 operands / f32 PSUM:
     A^T = K^T Q per head (heads grouped by partition parity per PSUM bank --
     a PE tile-row config may not change within a bank) -> masked-copy (DVE,
     doubles as the PSUM->SBUF move) -> O = A_m V + Q S8, all 8 heads in one
     bank; den via tiny matmuls (masked-A^T . ones/32 + q . z8); state
     [S | z/32] lives PERSISTENTLY in one PSUM bank (start only at chunk 0)
     with a per-chunk bf16 snapshot for the next chunk's Q S / q.z reads.
     den' = max(den, eps/32)*8 restores every scale factor exactly.
  3. PE-transpose o (bf16) -> output projection in bf16 -> bf16 partials
     DMA'd out; the host upcasts and sums core pairs in f32.

The emission order is software-pipelined: attention chunks of super-tile s
are interleaved with the projection of super-tile s+1 so the PE never waits
on the phi-combine consumer chain (engines execute their streams in order).
"""
import sys

sys.path.insert(0, "/opt/trn_rl_repo")
import numpy as np
import ml_dtypes

F8 = ml_dtypes.float8_e4m3
BFNP = ml_dtypes.bfloat16

D = 1024
L = 4096
B = 4
H = 16
DH = 64
FPC = 512          # features per core (8 heads x 64)
C = 128            # scan chunk
ST = 512           # super-tile tokens
NCH = ST // C      # 4
NST = L // ST      # 8
EPS = 1e-6

_CACHE = {}


def _build_nc():
    import concourse.bacc as bacc
    import concourse.mybir as mybir
    import concourse.tile as tile

    dt = mybir.dt
    f32, fp8, bf16 = dt.float32, dt.float8e4, dt.bfloat16
    Alu = mybir.AluOpType
    Act = mybir.ActivationFunctionType
    PM = mybir.MatmulPerfMode

    nc = bacc.Bacc("TRN2", target_bir_lowering=False, debug=True)

    xhi_d = nc.dram_tensor("xhi", [128, 4, 2, L], fp8, kind="ExternalInput")
    xlo_d = nc.dram_tensor("xlo", [128, 4, 2, L], fp8, kind="ExternalInput")
    wqk_d = nc.dram_tensor("wqk", [128, 8, 4, 2, 128], fp8, kind="ExternalInput")
    wvh_d = nc.dram_tensor("wvh", [128, 4, 2, FPC], fp8, kind="ExternalInput")
    wvl_d = nc.dram_tensor("wvl", [128, 4, 2, FPC], fp8, kind="ExternalInput")
    w2_d = nc.dram_tensor("w2", [128, 4, 8, 128], bf16, kind="ExternalInput")
    maskT_d = nc.dram_tensor("maskT", [C, C], f32, kind="ExternalInput")
    id128_d = nc.dram_tensor("id128", [128, 128], bf16, kind="ExternalInput")
    outT_d = nc.dram_tensor("outT", [D, L], bf16, kind="ExternalOutput")

    from contextlib import ExitStack

    with tile.TileContext(nc) as tc, ExitStack() as es:
        if True:
            wp = es.enter_context(tc.tile_pool(name="wp", bufs=1))
            xp = es.enter_context(tc.tile_pool(name="xp", bufs=2))
            qp = es.enter_context(tc.tile_pool(name="qp", bufs=4))
            scr = es.enter_context(tc.tile_pool(name="scr", bufs=3))
            vp = es.enter_context(tc.tile_pool(name="vp", bufs=2))
            kp = es.enter_context(tc.tile_pool(name="kp", bufs=3))
            atsp = es.enter_context(tc.tile_pool(name="atsp", bufs=3))
            szsp = es.enter_context(tc.tile_pool(name="szsp", bufs=3))
            dp = es.enter_context(tc.tile_pool(name="dp", bufs=4))
            ocp = es.enter_context(tc.tile_pool(name="ocp", bufs=3))
            osp = es.enter_context(tc.tile_pool(name="osp", bufs=2))
            o3p = es.enter_context(tc.tile_pool(name="o3p", bufs=2))
            projp = es.enter_context(tc.tile_pool(name="projp", bufs=2, space="PSUM"))
            atp = es.enter_context(tc.tile_pool(name="atp", bufs=2, space="PSUM"))
            obp = es.enter_context(tc.tile_pool(name="obp", bufs=1, space="PSUM"))
            szp = es.enter_context(tc.tile_pool(name="szp", bufs=1, space="PSUM"))
            miscp = es.enter_context(tc.tile_pool(name="miscp", bufs=1, space="PSUM"))
            trpp = es.enter_context(tc.tile_pool(name="trpp", bufs=1, space="PSUM"))

            # ---- resident constants / weights ----
            wqk = wp.tile([128, 8, 4, 2, 128], fp8, tag="wqk")
            nc.gpsimd.dma_start(wqk[:], wqk_d[:])
            wvh = wp.tile([128, 4, 2, FPC], fp8, tag="wvh")
            nc.scalar.dma_start(wvh[:], wvh_d[:])
            wvl = wp.tile([128, 4, 2, FPC], fp8, tag="wvl")
            nc.scalar.dma_start(wvl[:], wvl_d[:])
            w2 = wp.tile([128, 4, 8, 128], bf16, tag="w2")
            nc.scalar.dma_start(w2[:], w2_d[:])
            maskT = wp.tile([C, C], f32, tag="maskT")
            nc.scalar.dma_start(maskT[:], maskT_d[:])
            id128 = wp.tile([128, 128], bf16, tag="id128")
            nc.scalar.dma_start(id128[:], id128_d[:])
            wb8 = wp.tile([128, 2, 128], fp8, tag="wb8")
            nc.vector.memset(wb8[:], 0.0)
            nc.vector.memset(wb8[0:1, 0, :], 32.0)
            xb8 = wp.tile([128, 2, ST], fp8, tag="xb8")
            nc.vector.memset(xb8[:], 0.0)
            nc.vector.memset(xb8[0:1, 0, :], 1.0)
            ones32 = wp.tile([128, 1], bf16, tag="ones32")
            nc.vector.memset(ones32[:], 1.0 / 32.0)
            bm1 = wp.tile([128, 1], f32, tag="bm1")
            nc.vector.memset(bm1[:], float(np.log(32.0) - 1.0))

            # persistent scan state [S | z/32] per head pair, in PSUM.
            # Full-bank tile (2048B row pitch) so pending-zero bookkeeping
            # stays row-aligned for base_partition=64 matmul outputs.
            Sz_t = szp.tile([128, 512], f32, tag="Sz")
            Sz = Sz_t[:, 0:260].rearrange("p (j e) -> p j e", e=DH + 1)
            sz8_init = szsp.tile([128, 4, DH + 1], bf16, tag="Sz8", name="sz8i")
            nc.vector.memset(sz8_init[:], 0.0)
            szh = [sz8_init]  # rolling snapshot holder

            def emit_proj_tiles(st):
                t0 = st * ST
                xhi = xp.tile([128, 4, 2, ST], fp8, tag="xhi", name=f"xhi{st}")
                nc.sync.dma_start(xhi[:], xhi_d[:, :, :, t0 : t0 + ST])
                xlo = xp.tile([128, 4, 2, ST], fp8, tag="xlo", name=f"xlo{st}")
                nc.sync.dma_start(xlo[:], xlo_d[:, :, :, t0 : t0 + ST])
                qTb = qp.tile([128, 4, ST], bf16, tag="qTb", name=f"qTb{st}")
                kTb = qp.tile([128, 4, ST], bf16, tag="kTb", name=f"kTb{st}")
                v1 = vp.tile([128, NCH, 8, DH], bf16, tag="v1", name=f"v1{st}")
                return dict(xhi=xhi, xlo=xlo, qTb=qTb, kTb=kTb, v1=v1)

            def emit_fc(T, fc):
                pq = projp.tile([128, ST], f32, tag="proj", name=f"pq_{fc}")
                for hh in range(2):
                    sl = slice(256 * hh, 256 * hh + 256)
                    for j in range(4):
                        nc.tensor.matmul(
                            pq[:, sl],
                            wqk[:, fc, j, :, :],
                            T["xhi"][:, j, :, sl],
                            start=(hh == 0 and j == 0),
                            stop=False,
                            perf_mode=PM.DoubleRow,
                            skip_group_check=True,
                        )
                    nc.tensor.matmul(
                        pq[:, sl],
                        wb8[:],
                        xb8[:, :, sl],
                        start=False,
                        stop=(hh == 1),
                        perf_mode=PM.DoubleRow,
                        skip_group_check=True,
                    )
                texp = scr.tile([128, ST], f32, tag="texp", name=f"texp{fc}")
                nc.scalar.activation(
                    texp[:], pq[:], Act.Exp, bias=bm1[:], scale=1.0 / 32.0
                )
                dst = T["qTb"][:, fc, :] if fc < 4 else T["kTb"][:, fc - 4, :]
                nc.vector.scalar_tensor_tensor(
                    dst, texp[:], 32.0, pq[:], Alu.min, Alu.max
                )

            def emit_v(T, tcc):
                pv = projp.tile([128, FPC], f32, tag="proj", name=f"pv{tcc}")
                xc_hi = T["xhi"][:, :, :, tcc * 128 : (tcc + 1) * 128]
                xc_lo = T["xlo"][:, :, :, tcc * 128 : (tcc + 1) * 128]
                n = 0
                for hh in range(2):
                    sl = slice(256 * hh, 256 * hh + 256)
                    for xa, wv in ((xc_hi, wvh), (xc_hi, wvl), (xc_lo, wvh)):
                        for j in range(4):
                            n += 1
                            nc.tensor.matmul(
                                pv[:, sl],
                                xa[:, j, :, :],
                                wv[:, j, :, sl],
                                start=(n == 1),
                                stop=(n == 24),
                                perf_mode=PM.DoubleRow,
                                skip_group_check=True,
                            )
                nc.scalar.mul(
                    T["v1"][:, tcc, :, :],
                    pv[:].rearrange("p (h e) -> p h e", e=DH),
                    1.0 / 128.0,
                )

            def emit_chunk(T, st, tcc, outT_sb):
                cg = st * NCH + tcc
                tsl = slice(tcc * 128, (tcc + 1) * 128)
                qTb, kTb, v1 = T["qTb"], T["kTb"], T["v1"]
                sz8_prev = szh[0]
                misc = miscp.tile([128, 264], f32, tag="misc", name=f"misc{cg}")
                ktr = misc[:, 0:256].bitcast(bf16).rearrange(
                    "p (j t) -> p j t", j=4
                )  # [128, 4, 128] bf16
                den = misc[:, 256:264]  # [128, 8] f32
                trp_t = trpp.tile([128, 256], f32, tag="trp", name=f"trp{cg}")
                trp = trp_t[:].bitcast(bf16).rearrange(
                    "p (g t) -> p g t", g=4
                )  # [128, 4, 128] bf16

                # -- k transpose to token-major --
                for j in range(4):
                    nc.tensor.matmul(
                        ktr[:, j, :],
                        kTb[:, j, tsl],
                        id128[:],
                        is_transpose=True,
                        start=(j == 0),
                        stop=(j == 3),
                        skip_group_check=True,
                    )
                ksb = kp.tile([128, 8, DH], bf16, tag="ksb", name=f"ksb{cg}")
                nc.scalar.copy(
                    ksb[:], ktr[:].rearrange("p j (hh e) -> p (j hh) e", e=DH)
                )

                # -- A^T = K^T Q; bank g = heads of parity g (a PE tile-row
                # config may not change within a PSUM bank) --
                ATp = [
                    atp.tile([128, 4, C], f32, tag="ATp", name=f"atp{cg}_{g}")
                    for g in range(2)
                ]
                for h in range(8):
                    po = 64 * (h % 2)
                    fq = h // 2
                    nc.tensor.matmul(
                        ATp[h % 2][:, h // 2, :],
                        kTb[po : po + 64, fq, tsl],
                        qTb[po : po + 64, fq, tsl],
                        start=(h // 2 == 0),
                        stop=(h // 2 == 3),
                        skip_group_check=True,
                    )
                ATs = atsp.tile([128, 8, C], bf16, tag="ATs", name=f"ats{cg}")
                for g in range(2):
                    nc.vector.tensor_tensor(
                        ATs[:, 4 * g : 4 * g + 4, :],
                        ATp[g][:],
                        maskT[:].unsqueeze(1).broadcast_to([C, 4, C]),
                        Alu.mult,
                    )

                # -- O = A_m V + Q S8 (one bank, 8 heads) --
                Ob = obp.tile([128, 8, DH], f32, tag="Ob", name=f"ob{cg}")
                for h in range(8):
                    po = 64 * (h % 2)
                    fq = h // 2
                    nc.tensor.matmul(
                        Ob[:, h, :],
                        ATs[:, 4 * (h % 2) + h // 2, :],
                        v1[:, tcc, h, :],
                        start=(h == 0),
                        stop=False,
                        skip_group_check=True,
                    )
                    nc.tensor.matmul(
                        Ob[:, h, :],
                        qTb[po : po + 64, fq, tsl],
                        sz8_prev[po : po + 64, h // 2, 0:DH],
                        start=False,
                        stop=(h == 7),
                        skip_group_check=True,
                    )

                # -- den = A_m^T ones/32 + q . z8 --
                for h in range(8):
                    po = 64 * (h % 2)
                    fq = h // 2
                    nc.tensor.matmul(
                        den[:, h : h + 1],
                        ATs[:, 4 * (h % 2) + h // 2, :],
                        ones32[:],
                        start=(h == 0),
                        stop=False,
                        skip_group_check=True,
                    )
                    nc.tensor.matmul(
                        den[:, h : h + 1],
                        qTb[po : po + 64, fq, tsl],
                        sz8_prev[po : po + 64, h // 2, DH : DH + 1],
                        start=False,
                        stop=(h == 7),
                        skip_group_check=True,
                    )
                dens = dp.tile([C, 8], f32, tag="dens", name=f"dens{cg}")
                nc.vector.tensor_scalar(
                    dens[:], den[:], EPS / 32.0, 8.0, Alu.max, Alu.mult
                )
                rden = dp.tile([C, 8], f32, tag="rden", name=f"rden{cg}")
                nc.vector.reciprocal(rden[:], dens[:])

                # -- state update into the persistent Sz bank --
                for h in range(8):
                    po = 64 * (h % 2)
                    j = h // 2
                    nc.tensor.matmul(
                        Sz[po : po + 64, j, 0:DH],
                        ksb[:, h, :],
                        v1[:, tcc, h, :],
                        start=(cg == 0 and h <= 1),
                        stop=False,
                        skip_group_check=True,
                    )
                    nc.tensor.matmul(
                        Sz[po : po + 64, j, DH : DH + 1],
                        ksb[:, h, :],
                        ones32[:],
                        start=False,
                        stop=(cg == NST * NCH - 1 and h == 7),
                        skip_group_check=True,
                    )
                sz8_new = szsp.tile(
                    [128, 4, DH + 1], bf16, tag="Sz8", name=f"sz8_{cg}"
                )
                nc.scalar.copy(sz8_new[:], Sz[:])
                szh[0] = sz8_new

                # -- normalize -> bf16 o_c --
                oc8 = ocp.tile([C, 8, DH], bf16, tag="oc8", name=f"oc8{cg}")
                nc.vector.tensor_tensor(
                    oc8[:],
                    Ob[:],
                    rden[:].unsqueeze(2).broadcast_to([C, 8, DH]),
                    Alu.mult,
                )

                # -- transpose o_c to feature-major --
                oc_flat = oc8[:].rearrange("p h e -> p (h e)")
                for g in range(4):
                    nc.tensor.matmul(
                        trp[:, g, :],
                        oc_flat[:, g * 128 : (g + 1) * 128],
                        id128[:],
                        is_transpose=True,
                        start=(g == 0),
                        stop=(g == 3),
                        skip_group_check=True,
                    )
                nc.vector.tensor_copy(outT_sb[:, :, tsl], trp[:])

            def emit_outproj_part(st, outT_sb, o3sb, ocs, last):
                t0 = st * ST
                for oc in ocs:
                    po3 = projp.tile([128, ST], f32, tag="proj", name=f"po3_{oc}")
                    for j in range(4):
                        nc.tensor.matmul(
                            po3[:],
                            w2[:, j, oc, :],
                            outT_sb[:, j, :],
                            start=(j == 0),
                            stop=(j == 3),
                        )
                    nc.scalar.copy(o3sb[:, oc, :], po3[:])
                if last:
                    nc.sync.dma_start(
                        outT_d[:, t0 : t0 + ST].rearrange(
                            "(c p) t -> p c t", p=128
                        ),
                        o3sb[:],
                    )

            # ---- software-pipelined emission: chunks of st overlap the
            # projection of st+1 AND the output projection of st-1 ----
            tiles = {0: emit_proj_tiles(0)}
            for fc in range(8):
                emit_fc(tiles[0], fc)
            for tcc in range(NCH):
                emit_v(tiles[0], tcc)
            prev = None  # (st-1, outT_sb, o3sb)
            for st in range(NST):
                T = tiles.pop(st)
                Tn = None
                if st + 1 < NST:
                    Tn = emit_proj_tiles(st + 1)
                    tiles[st + 1] = Tn
                outT_sb = osp.tile(
                    [128, 4, ST], bf16, tag="outT_sb", name=f"osb{st}"
                )
                for tcc in range(NCH):
                    emit_chunk(T, st, tcc, outT_sb)
                    if prev is not None:
                        emit_outproj_part(
                            prev[0], prev[1], prev[2],
                            [2 * tcc, 2 * tcc + 1], last=(tcc == NCH - 1),
                        )
                    if Tn is not None:
                        emit_fc(Tn, 2 * tcc)
                        emit_fc(Tn, 2 * tcc + 1)
                        if tcc % 2 == 1:
                            emit_v(Tn, tcc - 1)
                            emit_v(Tn, tcc)
                o3sb = o3p.tile([128, 8, ST], bf16, tag="o3sb", name=f"o3sb{st}")
                prev = (st, outT_sb, o3sb)
            emit_outproj_part(prev[0], prev[1], prev[2], list(range(8)), last=True)

    nc.finalize()
    return nc


def _get_nc():
    if "nc" not in _CACHE:
        _CACHE["nc"] = _build_nc()
    return _CACHE["nc"]


def _pack_x(xT):
    """xT: [1024, 4096] f32 -> hi/lo fp8 in [128, 4, 2, L] layout."""
    xr = xT.reshape(4, 2, 128, L).transpose(2, 0, 1, 3)  # [128, 4, 2, L]
    hi = xr.astype(F8)
    lo = (xr - hi.astype(np.float32)).astype(F8)
    return np.ascontiguousarray(hi), np.ascontiguousarray(lo)


def _make_in_maps(x, w_qkv, w_out):
    maskT = np.triu(np.ones((C, C), np.float32))  # maskT[j,i] = 1 if j <= i
    id128 = np.eye(128, dtype=np.float32)
    in_maps = []
    for core in range(8):
        b, g = core // 2, core % 2
        xT = np.ascontiguousarray(x[b].T).astype(np.float32)
        xhi, xlo = _pack_x(xT)
        # q/k weights x32 (fp8 subnormal avoidance), transposed
        W1 = np.concatenate(
            [
                w_qkv[512 * g : 512 * (g + 1)],
                w_qkv[1024 + 512 * g : 1024 + 512 * (g + 1)],
            ],
            axis=0,
        ).T.astype(np.float32)  # [1024 d, 1024 f]
        wqk = (32.0 * W1).reshape(4, 2, 128, 8, 128).transpose(2, 3, 0, 1, 4)
        wqk8 = np.ascontiguousarray(wqk).astype(F8)
        # v weights x32, hi/lo split
        Wv = (32.0 * w_qkv[2048 + 512 * g : 2048 + 512 * (g + 1)].T).astype(
            np.float32
        )  # [1024 d, 512 f]
        Wvr = Wv.reshape(4, 2, 128, FPC).transpose(2, 0, 1, 3)  # [128,4,2,512]
        wvh = Wvr.astype(F8)
        wvl = (Wvr - wvh.astype(np.float32)).astype(F8)
        # output projection weights [512 f, 1024 oc] -> [128, 4, 8, 128]
        W2 = w_out[:, 512 * g : 512 * (g + 1)].T.astype(np.float32)
        w2r = W2.reshape(4, 128, 8, 128).transpose(1, 0, 2, 3)
        in_maps.append(
            {
                "xhi": xhi,
                "xlo": np.ascontiguousarray(xlo),
                "wqk": wqk8,
                "wvh": np.ascontiguousarray(wvh),
                "wvl": np.ascontiguousarray(wvl),
                "w2": np.ascontiguousarray(w2r).astype(BFNP),
                "maskT": maskT,
                "id128": id128.astype(BFNP),
            }
        )
    return in_maps


def _run(inputs, trace=False):
    from concourse.bass_utils import run_bass_kernel_spmd

    nc = _get_nc()
    in_maps = _make_in_maps(inputs["x"], inputs["w_qkv"], inputs["w_out"])
    res = run_bass_kernel_spmd(nc, in_maps, core_ids=list(range(8)), trace=trace)
    out = np.empty((B, L, D), np.float32)
    for b in range(B):
        p0 = res.results[2 * b]["outT"].astype(np.float32)
        p1 = res.results[2 * b + 1]["outT"].astype(np.float32)
        out[b] = (p0 + p1).T
    return out, res


def kernel(x, w_qkv, w_out):
    out, _ = _run({"x": x, "w_qkv": w_qkv, "w_out": w_out})
    return out


# revision 17
# speedup vs baseline: 1.8972x; 1.0046x over previous
"""TRN2 Bass kernel for nn_LinearAttention (B=4, L=4096, D=1024, H=16, dh=64).

Sharding: core c = batch c//2, heads (c%2)*8..+8. Zero cross-core comms; the
two half-head partial output projections per batch are summed on the host.

Per-core pipeline (super-tiles of 512 tokens, scan chunks of 128):
  1. QKV projection:
     - q/k: fp8e4 DoubleRow matmuls (0.5 cyc/row, K=256 per call). Weights are
       scaled x32 on the host (plain w_qkv values sit in e4m3's subnormal
       range and quantize at 6-50% error; x32 moves them to normals). A bias
       ones-row (weight 32) makes PSUM hold 32(z+1); phi is exact in 2 passes
       via 32*phi(z) = max(min(32*e^z, 32), 32z+32):
       Act Exp(scale=1/32, bias=ln32-1) then one DVE scalar_tensor_tensor.
       The x32 on q/k cancels between numerator and denominator.
     - v: 3-term compensated fp8 DoubleRow (x_hi w_hi + x_hi w_lo + x_lo w_hi)
       giving ~bf16 accuracy at 0.75x bf16 matmul cost; the copy to bf16
       applies 1/128 (un-scales the x32 and pre-divides v by 4 for range).
  2. Chunked linear attention, bf16 operands / f32 PSUM:
     A^T = K^T Q per head (heads grouped by partition parity per PSUM bank --
     a PE tile-row config may not change within a bank) -> masked-copy (DVE,
     doubles as the PSUM->SBUF move) -> O = A_m V + Q S8, all 8 heads in one
     bank; den via tiny matmuls (masked-A^T . ones/32 + q . z8); state
     [S | z/32] lives PERSISTENTLY in one PSUM bank (start only at chunk 0)
     with a per-chunk bf16 snapshot for the next chunk's Q S / q.z reads.
     den' = max(den, eps/32)*8 restores every scale factor exactly.
  3. PE-transpose o (bf16) -> output projection in bf16 -> bf16 partials
     DMA'd out; the host upcasts and sums core pairs in f32.

The emission order is software-pipelined: attention chunks of super-tile s
are interleaved with the projection of super-tile s+1 so the PE never waits
on the phi-combine consumer chain (engines execute their streams in order).
"""
import sys

sys.path.insert(0, "/opt/trn_rl_repo")
import numpy as np
import ml_dtypes

F8 = ml_dtypes.float8_e4m3
BFNP = ml_dtypes.bfloat16

D = 1024
L = 4096
B = 4
H = 16
DH = 64
FPC = 512          # features per core (8 heads x 64)
C = 128            # scan chunk
ST = 512           # super-tile tokens
NCH = ST // C      # 4
NST = L // ST      # 8
EPS = 1e-6

_CACHE = {}


def _build_nc():
    import concourse.bacc as bacc
    import concourse.mybir as mybir
    import concourse.tile as tile

    dt = mybir.dt
    f32, fp8, bf16 = dt.float32, dt.float8e4, dt.bfloat16
    Alu = mybir.AluOpType
    Act = mybir.ActivationFunctionType
    PM = mybir.MatmulPerfMode

    nc = bacc.Bacc("TRN2", target_bir_lowering=False, debug=True)

    xhi_d = nc.dram_tensor("xhi", [128, 4, 2, L], fp8, kind="ExternalInput")
    xlo_d = nc.dram_tensor("xlo", [128, 4, 2, L], fp8, kind="ExternalInput")
    wqk_d = nc.dram_tensor("wqk", [128, 8, 4, 2, 128], fp8, kind="ExternalInput")
    wvh_d = nc.dram_tensor("wvh", [128, 4, 2, FPC], fp8, kind="ExternalInput")
    wvl_d = nc.dram_tensor("wvl", [128, 4, 2, FPC], fp8, kind="ExternalInput")
    w2_d = nc.dram_tensor("w2", [128, 4, 8, 128], bf16, kind="ExternalInput")
    maskT_d = nc.dram_tensor("maskT", [C, C], f32, kind="ExternalInput")
    id128_d = nc.dram_tensor("id128", [128, 128], bf16, kind="ExternalInput")
    outT_d = nc.dram_tensor("outT", [D, L], bf16, kind="ExternalOutput")

    from contextlib import ExitStack

    with tile.TileContext(nc) as tc, ExitStack() as es:
        if True:
            wp = es.enter_context(tc.tile_pool(name="wp", bufs=1))
            xp = es.enter_context(tc.tile_pool(name="xp", bufs=2))
            qp = es.enter_context(tc.tile_pool(name="qp", bufs=4))
            scr = es.enter_context(tc.tile_pool(name="scr", bufs=3))
            vp = es.enter_context(tc.tile_pool(name="vp", bufs=2))
            kp = es.enter_context(tc.tile_pool(name="kp", bufs=3))
            atsp = es.enter_context(tc.tile_pool(name="atsp", bufs=3))
            szsp = es.enter_context(tc.tile_pool(name="szsp", bufs=3))
            dp = es.enter_context(tc.tile_pool(name="dp", bufs=4))
            ocp = es.enter_context(tc.tile_pool(name="ocp", bufs=3))
            osp = es.enter_context(tc.tile_pool(name="osp", bufs=2))
            o3p = es.enter_context(tc.tile_pool(name="o3p", bufs=2))
            projp = es.enter_context(tc.tile_pool(name="projp", bufs=2, space="PSUM"))
            atp = es.enter_context(tc.tile_pool(name="atp", bufs=2, space="PSUM"))
            obp = es.enter_context(tc.tile_pool(name="obp", bufs=1, space="PSUM"))
            szp = es.enter_context(tc.tile_pool(name="szp", bufs=1, space="PSUM"))
            miscp = es.enter_context(tc.tile_pool(name="miscp", bufs=1, space="PSUM"))
            trpp = es.enter_context(tc.tile_pool(name="trpp", bufs=1, space="PSUM"))

            # ---- resident constants / weights ----
            wqk = wp.tile([128, 8, 4, 2, 128], fp8, tag="wqk")
            nc.scalar.dma_start(wqk[:], wqk_d[:])
            wvh = wp.tile([128, 4, 2, FPC], fp8, tag="wvh")
            nc.scalar.dma_start(wvh[:], wvh_d[:])
            wvl = wp.tile([128, 4, 2, FPC], fp8, tag="wvl")
            nc.scalar.dma_start(wvl[:], wvl_d[:])
            w2 = wp.tile([128, 4, 8, 128], bf16, tag="w2")
            nc.gpsimd.dma_start(w2[:], w2_d[:])
            maskT = wp.tile([C, C], f32, tag="maskT")
            nc.gpsimd.dma_start(maskT[:], maskT_d[:])
            id128 = wp.tile([128, 128], bf16, tag="id128")
            nc.gpsimd.dma_start(id128[:], id128_d[:])
            wb8 = wp.tile([128, 2, 128], fp8, tag="wb8")
            nc.vector.memset(wb8[:], 0.0)
            nc.vector.memset(wb8[0:1, 0, :], 32.0)
            xb8 = wp.tile([128, 2, ST], fp8, tag="xb8")
            nc.vector.memset(xb8[:], 0.0)
            nc.vector.memset(xb8[0:1, 0, :], 1.0)
            ones32 = wp.tile([128, 1], bf16, tag="ones32")
            nc.vector.memset(ones32[:], 1.0 / 32.0)
            bm1 = wp.tile([128, 1], f32, tag="bm1")
            nc.vector.memset(bm1[:], float(np.log(32.0) - 1.0))

            # persistent scan state [S | z/32] per head pair, in PSUM.
            # Full-bank tile (2048B row pitch) so pending-zero bookkeeping
            # stays row-aligned for base_partition=64 matmul outputs.
            Sz_t = szp.tile([128, 512], f32, tag="Sz")
            Sz = Sz_t[:, 0:260].rearrange("p (j e) -> p j e", e=DH + 1)
            sz8_init = szsp.tile([128, 4, DH + 1], bf16, tag="Sz8", name="sz8i")
            nc.vector.memset(sz8_init[:], 0.0)
            szh = [sz8_init]  # rolling snapshot holder

            def emit_proj_tiles(st):
                t0 = st * ST
                xhi = xp.tile([128, 4, 2, ST], fp8, tag="xhi", name=f"xhi{st}")
                nc.sync.dma_start(xhi[:], xhi_d[:, :, :, t0 : t0 + ST])
                xlo = xp.tile([128, 4, 2, ST], fp8, tag="xlo", name=f"xlo{st}")
                nc.sync.dma_start(xlo[:], xlo_d[:, :, :, t0 : t0 + ST])
                qTb = qp.tile([128, 4, ST], bf16, tag="qTb", name=f"qTb{st}")
                kTb = qp.tile([128, 4, ST], bf16, tag="kTb", name=f"kTb{st}")
                v1 = vp.tile([128, NCH, 8, DH], bf16, tag="v1", name=f"v1{st}")
                return dict(xhi=xhi, xlo=xlo, qTb=qTb, kTb=kTb, v1=v1)

            def emit_fc(T, fc):
                pq = projp.tile([128, ST], f32, tag="proj", name=f"pq_{fc}")
                for hh in range(2):
                    sl = slice(256 * hh, 256 * hh + 256)
                    for j in range(4):
                        nc.tensor.matmul(
                            pq[:, sl],
                            wqk[:, fc, j, :, :],
                            T["xhi"][:, j, :, sl],
                            start=(hh == 0 and j == 0),
                            stop=False,
                            perf_mode=PM.DoubleRow,
                            skip_group_check=True,
                        )
                    nc.tensor.matmul(
                        pq[:, sl],
                        wb8[:],
                        xb8[:, :, sl],
                        start=False,
                        stop=(hh == 1),
                        perf_mode=PM.DoubleRow,
                        skip_group_check=True,
                    )
                texp = scr.tile([128, ST], f32, tag="texp", name=f"texp{fc}")
                nc.scalar.activation(
                    texp[:], pq[:], Act.Exp, bias=bm1[:], scale=1.0 / 32.0
                )
                dst = T["qTb"][:, fc, :] if fc < 4 else T["kTb"][:, fc - 4, :]
                nc.vector.scalar_tensor_tensor(
                    dst, texp[:], 32.0, pq[:], Alu.min, Alu.max
                )

            def emit_v(T, tcc):
                pv = projp.tile([128, FPC], f32, tag="proj", name=f"pv{tcc}")
                xc_hi = T["xhi"][:, :, :, tcc * 128 : (tcc + 1) * 128]
                xc_lo = T["xlo"][:, :, :, tcc * 128 : (tcc + 1) * 128]
                n = 0
                for hh in range(2):
                    sl = slice(256 * hh, 256 * hh + 256)
                    for xa, wv in ((xc_hi, wvh), (xc_hi, wvl), (xc_lo, wvh)):
                        for j in range(4):
                            n += 1
                            nc.tensor.matmul(
                                pv[:, sl],
                                xa[:, j, :, :],
                                wv[:, j, :, sl],
                                start=(n == 1),
                                stop=(n == 24),
                                perf_mode=PM.DoubleRow,
                                skip_group_check=True,
                            )
                nc.scalar.mul(
                    T["v1"][:, tcc, :, :],
                    pv[:].rearrange("p (h e) -> p h e", e=DH),
                    1.0 / 128.0,
                )

            def emit_chunk(T, st, tcc, outT_sb):
                cg = st * NCH + tcc
                tsl = slice(tcc * 128, (tcc + 1) * 128)
                qTb, kTb, v1 = T["qTb"], T["kTb"], T["v1"]
                sz8_prev = szh[0]
                misc = miscp.tile([128, 264], f32, tag="misc", name=f"misc{cg}")
                ktr = misc[:, 0:256].bitcast(bf16).rearrange(
                    "p (j t) -> p j t", j=4
                )  # [128, 4, 128] bf16
                den = misc[:, 256:264]  # [128, 8] f32
                trp_t = trpp.tile([128, 256], f32, tag="trp", name=f"trp{cg}")
                trp = trp_t[:].bitcast(bf16).rearrange(
                    "p (g t) -> p g t", g=4
                )  # [128, 4, 128] bf16

                # -- k transpose to token-major --
                for j in range(4):
                    nc.tensor.matmul(
                        ktr[:, j, :],
                        kTb[:, j, tsl],
                        id128[:],
                        is_transpose=True,
                        start=(j == 0),
                        stop=(j == 3),
                        skip_group_check=True,
                    )
                ksb = kp.tile([128, 8, DH], bf16, tag="ksb", name=f"ksb{cg}")
                nc.scalar.copy(
                    ksb[:], ktr[:].rearrange("p j (hh e) -> p (j hh) e", e=DH)
                )

                # -- A^T = K^T Q; bank g = heads of parity g (a PE tile-row
                # config may not change within a PSUM bank) --
                ATp = [
                    atp.tile([128, 4, C], f32, tag="ATp", name=f"atp{cg}_{g}")
                    for g in range(2)
                ]
                for h in range(8):
                    po = 64 * (h % 2)
                    fq = h // 2
                    nc.tensor.matmul(
                        ATp[h % 2][:, h // 2, :],
                        kTb[po : po + 64, fq, tsl],
                        qTb[po : po + 64, fq, tsl],
                        start=(h // 2 == 0),
                        stop=(h // 2 == 3),
                        skip_group_check=True,
                    )
                ATs = atsp.tile([128, 8, C], bf16, tag="ATs", name=f"ats{cg}")
                for g in range(2):
                    nc.vector.tensor_tensor(
                        ATs[:, 4 * g : 4 * g + 4, :],
                        ATp[g][:],
                        maskT[:].unsqueeze(1).broadcast_to([C, 4, C]),
                        Alu.mult,
                    )

                # -- O = A_m V + Q S8 (one bank, 8 heads) --
                Ob = obp.tile([128, 8, DH], f32, tag="Ob", name=f"ob{cg}")
                for h in range(8):
                    po = 64 * (h % 2)
                    fq = h // 2
                    nc.tensor.matmul(
                        Ob[:, h, :],
                        ATs[:, 4 * (h % 2) + h // 2, :],
                        v1[:, tcc, h, :],
                        start=(h == 0),
                        stop=False,
                        skip_group_check=True,
                    )
                    nc.tensor.matmul(
                        Ob[:, h, :],
                        qTb[po : po + 64, fq, tsl],
                        sz8_prev[po : po + 64, h // 2, 0:DH],
                        start=False,
                        stop=(h == 7),
                        skip_group_check=True,
                    )

                # -- den = A_m^T ones/32 + q . z8 --
                for h in range(8):
                    po = 64 * (h % 2)
                    fq = h // 2
                    nc.tensor.matmul(
                        den[:, h : h + 1],
                        ATs[:, 4 * (h % 2) + h // 2, :],
                        ones32[:],
                        start=(h == 0),
                        stop=False,
                        skip_group_check=True,
                    )
                    nc.tensor.matmul(
                        den[:, h : h + 1],
                        qTb[po : po + 64, fq, tsl],
                        sz8_prev[po : po + 64, h // 2, DH : DH + 1],
                        start=False,
                        stop=(h == 7),
                        skip_group_check=True,
                    )
                dens = dp.tile([C, 8], f32, tag="dens", name=f"dens{cg}")
                nc.vector.tensor_scalar(
                    dens[:], den[:], EPS / 32.0, 8.0, Alu.max, Alu.mult
                )
                rden = dp.tile([C, 8], f32, tag="rden", name=f"rden{cg}")
                nc.vector.reciprocal(rden[:], dens[:])

                # -- state update into the persistent Sz bank --
                for h in range(8):
                    po = 64 * (h % 2)
                    j = h // 2
                    nc.tensor.matmul(
                        Sz[po : po + 64, j, 0:DH],
                        ksb[:, h, :],
                        v1[:, tcc, h, :],
                        start=(cg == 0 and h <= 1),
                        stop=False,
                        skip_group_check=True,
                    )
                    nc.tensor.matmul(
                        Sz[po : po + 64, j, DH : DH + 1],
                        ksb[:, h, :],
                        ones32[:],
                        start=False,
                        stop=(cg == NST * NCH - 1 and h == 7),
                        skip_group_check=True,
                    )
                sz8_new = szsp.tile(
                    [128, 4, DH + 1], bf16, tag="Sz8", name=f"sz8_{cg}"
                )
                nc.scalar.copy(sz8_new[:], Sz[:])
                szh[0] = sz8_new

                # -- normalize -> bf16 o_c --
                oc8 = ocp.tile([C, 8, DH], bf16, tag="oc8", name=f"oc8{cg}")
                nc.vector.tensor_tensor(
                    oc8[:],
                    Ob[:],
                    rden[:].unsqueeze(2).broadcast_to([C, 8, DH]),
                    Alu.mult,
                )

                # -- transpose o_c to feature-major --
                oc_flat = oc8[:].rearrange("p h e -> p (h e)")
                for g in range(4):
                    nc.tensor.matmul(
                        trp[:, g, :],
                        oc_flat[:, g * 128 : (g + 1) * 128],
                        id128[:],
                        is_transpose=True,
                        start=(g == 0),
                        stop=(g == 3),
                        skip_group_check=True,
                    )
                nc.vector.tensor_copy(outT_sb[:, :, tsl], trp[:])

            def emit_outproj_part(st, outT_sb, o3sb, ocs, last):
                t0 = st * ST
                for oc in ocs:
                    po3 = projp.tile([128, ST], f32, tag="proj", name=f"po3_{oc}")
                    for j in range(4):
                        nc.tensor.matmul(
                            po3[:],
                            w2[:, j, oc, :],
                            outT_sb[:, j, :],
                            start=(j == 0),
                            stop=(j == 3),
                        )
                    nc.scalar.copy(o3sb[:, oc, :], po3[:])
                if last:
                    nc.sync.dma_start(
                        outT_d[:, t0 : t0 + ST].rearrange(
                            "(c p) t -> p c t", p=128
                        ),
                        o3sb[:],
                    )

            # ---- software-pipelined emission: chunks of st overlap the
            # projection of st+1 AND the output projection of st-1 ----
            tiles = {0: emit_proj_tiles(0)}
            for fc in range(8):
                emit_fc(tiles[0], fc)
            for tcc in range(NCH):
                emit_v(tiles[0], tcc)
            prev = None  # (st-1, outT_sb, o3sb)
            for st in range(NST):
                T = tiles.pop(st)
                Tn = None
                if st + 1 < NST:
                    Tn = emit_proj_tiles(st + 1)
                    tiles[st + 1] = Tn
                outT_sb = osp.tile(
                    [128, 4, ST], bf16, tag="outT_sb", name=f"osb{st}"
                )
                for tcc in range(NCH):
                    emit_chunk(T, st, tcc, outT_sb)
                    if prev is not None:
                        emit_outproj_part(
                            prev[0], prev[1], prev[2],
                            [2 * tcc, 2 * tcc + 1], last=(tcc == NCH - 1),
                        )
                    if Tn is not None:
                        emit_fc(Tn, 2 * tcc)
                        emit_fc(Tn, 2 * tcc + 1)
                        if tcc % 2 == 1:
                            emit_v(Tn, tcc - 1)
                            emit_v(Tn, tcc)
                o3sb = o3p.tile([128, 8, ST], bf16, tag="o3sb", name=f"o3sb{st}")
                prev = (st, outT_sb, o3sb)
            emit_outproj_part(prev[0], prev[1], prev[2], list(range(8)), last=True)

    nc.finalize()
    return nc


def _get_nc():
    if "nc" not in _CACHE:
        _CACHE["nc"] = _build_nc()
    return _CACHE["nc"]


def _pack_x(xT):
    """xT: [1024, 4096] f32 -> hi/lo fp8 in [128, 4, 2, L] layout."""
    xr = xT.reshape(4, 2, 128, L).transpose(2, 0, 1, 3)  # [128, 4, 2, L]
    hi = xr.astype(F8)
    lo = (xr - hi.astype(np.float32)).astype(F8)
    return np.ascontiguousarray(hi), np.ascontiguousarray(lo)


def _make_in_maps(x, w_qkv, w_out):
    maskT = np.triu(np.ones((C, C), np.float32))  # maskT[j,i] = 1 if j <= i
    id128 = np.eye(128, dtype=np.float32)
    in_maps = []
    for core in range(8):
        b, g = core // 2, core % 2
        xT = np.ascontiguousarray(x[b].T).astype(np.float32)
        xhi, xlo = _pack_x(xT)
        # q/k weights x32 (fp8 subnormal avoidance), transposed
        W1 = np.concatenate(
            [
                w_qkv[512 * g : 512 * (g + 1)],
                w_qkv[1024 + 512 * g : 1024 + 512 * (g + 1)],
            ],
            axis=0,
        ).T.astype(np.float32)  # [1024 d, 1024 f]
        wqk = (32.0 * W1).reshape(4, 2, 128, 8, 128).transpose(2, 3, 0, 1, 4)
        wqk8 = np.ascontiguousarray(wqk).astype(F8)
        # v weights x32, hi/lo split
        Wv = (32.0 * w_qkv[2048 + 512 * g : 2048 + 512 * (g + 1)].T).astype(
            np.float32
        )  # [1024 d, 512 f]
        Wvr = Wv.reshape(4, 2, 128, FPC).transpose(2, 0, 1, 3)  # [128,4,2,512]
        wvh = Wvr.astype(F8)
        wvl = (Wvr - wvh.astype(np.float32)).astype(F8)
        # output projection weights [512 f, 1024 oc] -> [128, 4, 8, 128]
        W2 = w_out[:, 512 * g : 512 * (g + 1)].T.astype(np.float32)
        w2r = W2.reshape(4, 128, 8, 128).transpose(1, 0, 2, 3)
        in_maps.append(
            {
                "xhi": xhi,
                "xlo": np.ascontiguousarray(xlo),
                "wqk": wqk8,
                "wvh": np.ascontiguousarray(wvh),
                "wvl": np.ascontiguousarray(wvl),
                "w2": np.ascontiguousarray(w2r).astype(BFNP),
                "maskT": maskT,
                "id128": id128.astype(BFNP),
            }
        )
    return in_maps


def _run(inputs, trace=False):
    from concourse.bass_utils import run_bass_kernel_spmd

    nc = _get_nc()
    in_maps = _make_in_maps(inputs["x"], inputs["w_qkv"], inputs["w_out"])
    res = run_bass_kernel_spmd(nc, in_maps, core_ids=list(range(8)), trace=trace)
    out = np.empty((B, L, D), np.float32)
    for b in range(B):
        p0 = res.results[2 * b]["outT"].astype(np.float32)
        p1 = res.results[2 * b + 1]["outT"].astype(np.float32)
        out[b] = (p0 + p1).T
    return out, res


def kernel(x, w_qkv, w_out):
    out, _ = _run({"x": x, "w_qkv": w_qkv, "w_out": w_out})
    return out


# revision 21
# speedup vs baseline: 1.9399x; 1.0225x over previous
"""TRN2 Bass kernel for nn_LinearAttention (B=4, L=4096, D=1024, H=16, dh=64).

Sharding: core c = batch c//2, heads (c%2)*8..+8. Zero cross-core comms; the
two half-head partial output projections per batch are summed on the host.

Per-core pipeline (super-tiles of 512 tokens, scan chunks of 128):
  1. QKV projection:
     - q/k: fp8e4 DoubleRow matmuls (0.5 cyc/row, K=256 per call). Weights are
       scaled x32 on the host (plain w_qkv values sit in e4m3's subnormal
       range and quantize at 6-50% error; x32 moves them to normals). A bias
       ones-row (weight 32) makes PSUM hold 32(z+1); phi is exact in 2 passes
       via 32*phi(z) = max(min(32*e^z, 32), 32z+32):
       Act Exp(scale=1/32, bias=ln32-1) then one DVE scalar_tensor_tensor.
       The x32 on q/k cancels between numerator and denominator.
     - v: 3-term compensated fp8 DoubleRow (x_hi w_hi + x_hi w_lo + x_lo w_hi)
       giving ~bf16 accuracy at 0.75x bf16 matmul cost; the copy to bf16
       applies 1/128 (un-scales the x32 and pre-divides v by 4 for range).
  2. Chunked linear attention, bf16 operands / f32 PSUM:
     A^T = K^T Q per head (heads grouped by partition parity per PSUM bank --
     a PE tile-row config may not change within a bank) -> masked-copy (DVE,
     doubles as the PSUM->SBUF move) -> O = A_m V + Q S8, all 8 heads in one
     bank; den via tiny matmuls (masked-A^T . ones/32 + q . z8); state
     [S | z/32] lives PERSISTENTLY in one PSUM bank (start only at chunk 0)
     with a per-chunk bf16 snapshot for the next chunk's Q S / q.z reads.
     den' = max(den, eps/32)*8 restores every scale factor exactly.
  3. PE-transpose o (bf16) -> output projection in bf16 -> bf16 partials
     DMA'd out; the host upcasts and sums core pairs in f32.

The emission order is software-pipelined: attention chunks of super-tile s
are interleaved with the projection of super-tile s+1 so the PE never waits
on the phi-combine consumer chain (engines execute their streams in order).
"""
import sys

sys.path.insert(0, "/opt/trn_rl_repo")
import numpy as np
import ml_dtypes

F8 = ml_dtypes.float8_e4m3
BFNP = ml_dtypes.bfloat16

D = 1024
L = 4096
B = 4
H = 16
DH = 64
FPC = 512          # features per core (8 heads x 64)
C = 128            # scan chunk
ST = 512           # super-tile tokens
NCH = ST // C      # 4
NST = L // ST      # 8
EPS = 1e-6

_CACHE = {}


def _build_nc():
    import concourse.bacc as bacc
    import concourse.mybir as mybir
    import concourse.tile as tile

    dt = mybir.dt
    f32, fp8, bf16 = dt.float32, dt.float8e4, dt.bfloat16
    Alu = mybir.AluOpType
    Act = mybir.ActivationFunctionType
    PM = mybir.MatmulPerfMode

    nc = bacc.Bacc("TRN2", target_bir_lowering=False, debug=True)

    xhi_d = nc.dram_tensor("xhi", [128, 4, 2, L], fp8, kind="ExternalInput")
    xlo_d = nc.dram_tensor("xlo", [128, 4, 2, L], fp8, kind="ExternalInput")
    wqk_d = nc.dram_tensor("wqk", [128, 8, 4, 2, 128], fp8, kind="ExternalInput")
    wvh_d = nc.dram_tensor("wvh", [128, 4, 2, FPC], fp8, kind="ExternalInput")
    wvl_d = nc.dram_tensor("wvl", [128, 4, 2, FPC], fp8, kind="ExternalInput")
    w2_d = nc.dram_tensor("w2", [128, 4, 8, 128], bf16, kind="ExternalInput")
    maskT_d = nc.dram_tensor("maskT", [C, C], f32, kind="ExternalInput")
    id128_d = nc.dram_tensor("id128", [128, 128], bf16, kind="ExternalInput")
    outT_d = nc.dram_tensor("outT", [D, L], bf16, kind="ExternalOutput")

    from contextlib import ExitStack

    with tile.TileContext(nc) as tc, ExitStack() as es:
        if True:
            wp = es.enter_context(tc.tile_pool(name="wp", bufs=1))
            xp = es.enter_context(tc.tile_pool(name="xp", bufs=4))
            qp = es.enter_context(tc.tile_pool(name="qp", bufs=6))
            scr = es.enter_context(tc.tile_pool(name="scr", bufs=6))
            vp = es.enter_context(tc.tile_pool(name="vp", bufs=2))
            kp = es.enter_context(tc.tile_pool(name="kp", bufs=5))
            atsp = es.enter_context(tc.tile_pool(name="atsp", bufs=5))
            szsp = es.enter_context(tc.tile_pool(name="szsp", bufs=5))
            dp = es.enter_context(tc.tile_pool(name="dp", bufs=8))
            ocp = es.enter_context(tc.tile_pool(name="ocp", bufs=5))
            osp = es.enter_context(tc.tile_pool(name="osp", bufs=3))
            o3p = es.enter_context(tc.tile_pool(name="o3p", bufs=2))
            projp = es.enter_context(tc.tile_pool(name="projp", bufs=2, space="PSUM"))
            atp = es.enter_context(tc.tile_pool(name="atp", bufs=2, space="PSUM"))
            obp = es.enter_context(tc.tile_pool(name="obp", bufs=1, space="PSUM"))
            szp = es.enter_context(tc.tile_pool(name="szp", bufs=1, space="PSUM"))
            miscp = es.enter_context(tc.tile_pool(name="miscp", bufs=1, space="PSUM"))
            trpp = es.enter_context(tc.tile_pool(name="trpp", bufs=1, space="PSUM"))

            # ---- resident constants / weights ----
            wqk = wp.tile([128, 8, 4, 2, 128], fp8, tag="wqk")
            nc.scalar.dma_start(wqk[:], wqk_d[:])
            wvh = wp.tile([128, 4, 2, FPC], fp8, tag="wvh")
            nc.scalar.dma_start(wvh[:], wvh_d[:])
            wvl = wp.tile([128, 4, 2, FPC], fp8, tag="wvl")
            nc.scalar.dma_start(wvl[:], wvl_d[:])
            w2 = wp.tile([128, 4, 8, 128], bf16, tag="w2")
            nc.gpsimd.dma_start(w2[:], w2_d[:])
            maskT = wp.tile([C, C], f32, tag="maskT")
            nc.gpsimd.dma_start(maskT[:], maskT_d[:])
            id128 = wp.tile([128, 128], bf16, tag="id128")
            nc.gpsimd.dma_start(id128[:], id128_d[:])
            wb8 = wp.tile([128, 2, 128], fp8, tag="wb8")
            nc.vector.memset(wb8[:], 0.0)
            nc.vector.memset(wb8[0:1, 0, :], 32.0)
            xb8 = wp.tile([128, 2, ST], fp8, tag="xb8")
            nc.vector.memset(xb8[:], 0.0)
            nc.vector.memset(xb8[0:1, 0, :], 1.0)
            ones32 = wp.tile([128, 1], bf16, tag="ones32")
            nc.vector.memset(ones32[:], 1.0 / 32.0)
            bm1 = wp.tile([128, 1], f32, tag="bm1")
            nc.vector.memset(bm1[:], float(np.log(32.0) - 1.0))

            # persistent scan state [S | z/32] per head pair, in PSUM.
            # Full-bank tile (2048B row pitch) so pending-zero bookkeeping
            # stays row-aligned for base_partition=64 matmul outputs.
            Sz_t = szp.tile([128, 512], f32, tag="Sz")
            Sz = Sz_t[:, 0:260].rearrange("p (j e) -> p j e", e=DH + 1)
            sz8_init = szsp.tile([128, 4, DH + 1], bf16, tag="Sz8", name="sz8i")
            nc.vector.memset(sz8_init[:], 0.0)
            szh = [sz8_init]  # rolling snapshot holder

            def emit_proj_tiles(st):
                t0 = st * ST
                xhi = xp.tile([128, 4, 2, ST], fp8, tag="xhi", name=f"xhi{st}")
                nc.sync.dma_start(xhi[:], xhi_d[:, :, :, t0 : t0 + ST])
                xlo = xp.tile([128, 4, 2, ST], fp8, tag="xlo", name=f"xlo{st}")
                nc.sync.dma_start(xlo[:], xlo_d[:, :, :, t0 : t0 + ST])
                qTb = qp.tile([128, 4, ST], bf16, tag="qTb", name=f"qTb{st}")
                kTb = qp.tile([128, 4, ST], bf16, tag="kTb", name=f"kTb{st}")
                v1 = vp.tile([128, NCH, 8, DH], bf16, tag="v1", name=f"v1{st}")
                return dict(xhi=xhi, xlo=xlo, qTb=qTb, kTb=kTb, v1=v1)

            def emit_fc(T, fc):
                pq = projp.tile([128, ST], f32, tag="proj", name=f"pq_{fc}")
                for hh in range(2):
                    sl = slice(256 * hh, 256 * hh + 256)
                    for j in range(4):
                        nc.tensor.matmul(
                            pq[:, sl],
                            wqk[:, fc, j, :, :],
                            T["xhi"][:, j, :, sl],
                            start=(hh == 0 and j == 0),
                            stop=False,
                            perf_mode=PM.DoubleRow,
                            skip_group_check=True,
                        )
                    nc.tensor.matmul(
                        pq[:, sl],
                        wb8[:],
                        xb8[:, :, sl],
                        start=False,
                        stop=(hh == 1),
                        perf_mode=PM.DoubleRow,
                        skip_group_check=True,
                    )
                texp = scr.tile([128, ST], f32, tag="texp", name=f"texp{fc}")
                nc.scalar.activation(
                    texp[:], pq[:], Act.Exp, bias=bm1[:], scale=1.0 / 32.0
                )
                dst = T["qTb"][:, fc, :] if fc < 4 else T["kTb"][:, fc - 4, :]
                nc.vector.scalar_tensor_tensor(
                    dst, texp[:], 32.0, pq[:], Alu.min, Alu.max
                )

            def emit_v(T, tcc):
                pv = projp.tile([128, FPC], f32, tag="proj", name=f"pv{tcc}")
                xc_hi = T["xhi"][:, :, :, tcc * 128 : (tcc + 1) * 128]
                xc_lo = T["xlo"][:, :, :, tcc * 128 : (tcc + 1) * 128]
                n = 0
                for hh in range(2):
                    sl = slice(256 * hh, 256 * hh + 256)
                    for xa, wv in ((xc_hi, wvh), (xc_hi, wvl), (xc_lo, wvh)):
                        for j in range(4):
                            n += 1
                            nc.tensor.matmul(
                                pv[:, sl],
                                xa[:, j, :, :],
                                wv[:, j, :, sl],
                                start=(n == 1),
                                stop=(n == 24),
                                perf_mode=PM.DoubleRow,
                                skip_group_check=True,
                            )
                nc.scalar.mul(
                    T["v1"][:, tcc, :, :],
                    pv[:].rearrange("p (h e) -> p h e", e=DH),
                    1.0 / 128.0,
                )

            def emit_chunk(T, st, tcc, outT_sb):
                cg = st * NCH + tcc
                tsl = slice(tcc * 128, (tcc + 1) * 128)
                qTb, kTb, v1 = T["qTb"], T["kTb"], T["v1"]
                sz8_prev = szh[0]
                misc = miscp.tile([128, 264], f32, tag="misc", name=f"misc{cg}")
                ktr = misc[:, 0:256].bitcast(bf16).rearrange(
                    "p (j t) -> p j t", j=4
                )  # [128, 4, 128] bf16
                den = misc[:, 256:264]  # [128, 8] f32
                trp_t = trpp.tile([128, 256], f32, tag="trp", name=f"trp{cg}")
                trp = trp_t[:].bitcast(bf16).rearrange(
                    "p (g t) -> p g t", g=4
                )  # [128, 4, 128] bf16

                # -- k transpose to token-major --
                for j in range(4):
                    nc.tensor.matmul(
                        ktr[:, j, :],
                        kTb[:, j, tsl],
                        id128[:],
                        is_transpose=True,
                        start=(j == 0),
                        stop=(j == 3),
                        skip_group_check=True,
                    )
                ksb = kp.tile([128, 8, DH], bf16, tag="ksb", name=f"ksb{cg}")
                nc.scalar.copy(
                    ksb[:], ktr[:].rearrange("p j (hh e) -> p (j hh) e", e=DH)
                )

                # -- A^T = K^T Q; bank g = heads of parity g (a PE tile-row
                # config may not change within a PSUM bank) --
                ATp = [
                    atp.tile([128, 4, C], f32, tag="ATp", name=f"atp{cg}_{g}")
                    for g in range(2)
                ]
                for h in range(8):
                    po = 64 * (h % 2)
                    fq = h // 2
                    nc.tensor.matmul(
                        ATp[h % 2][:, h // 2, :],
                        kTb[po : po + 64, fq, tsl],
                        qTb[po : po + 64, fq, tsl],
                        start=(h // 2 == 0),
                        stop=(h // 2 == 3),
                        skip_group_check=True,
                    )
                ATs = atsp.tile([128, 8, C], bf16, tag="ATs", name=f"ats{cg}")
                for g in range(2):
                    nc.vector.tensor_tensor(
                        ATs[:, 4 * g : 4 * g + 4, :],
                        ATp[g][:],
                        maskT[:].unsqueeze(1).broadcast_to([C, 4, C]),
                        Alu.mult,
                    )

                # -- O = A_m V + Q S8 (one bank, 8 heads) --
                Ob = obp.tile([128, 8, DH], f32, tag="Ob", name=f"ob{cg}")
                for h in range(8):
                    po = 64 * (h % 2)
                    fq = h // 2
                    nc.tensor.matmul(
                        Ob[:, h, :],
                        ATs[:, 4 * (h % 2) + h // 2, :],
                        v1[:, tcc, h, :],
                        start=(h == 0),
                        stop=False,
                        skip_group_check=True,
                    )
                    nc.tensor.matmul(
                        Ob[:, h, :],
                        qTb[po : po + 64, fq, tsl],
                        sz8_prev[po : po + 64, h // 2, 0:DH],
                        start=False,
                        stop=(h == 7),
                        skip_group_check=True,
                    )

                # -- den = A_m^T ones/32 + q . z8 --
                for h in range(8):
                    po = 64 * (h % 2)
                    fq = h // 2
                    nc.tensor.matmul(
                        den[:, h : h + 1],
                        ATs[:, 4 * (h % 2) + h // 2, :],
                        ones32[:],
                        start=(h == 0),
                        stop=False,
                        skip_group_check=True,
                    )
                    nc.tensor.matmul(
                        den[:, h : h + 1],
                        qTb[po : po + 64, fq, tsl],
                        sz8_prev[po : po + 64, h // 2, DH : DH + 1],
                        start=False,
                        stop=(h == 7),
                        skip_group_check=True,
                    )
                dens = dp.tile([C, 8], f32, tag="dens", name=f"dens{cg}")
                nc.vector.tensor_scalar(
                    dens[:], den[:], EPS / 32.0, 8.0, Alu.max, Alu.mult
                )
                rden = dp.tile([C, 8], f32, tag="rden", name=f"rden{cg}")
                nc.vector.reciprocal(rden[:], dens[:])

                # -- state update into the persistent Sz bank --
                for h in range(8):
                    po = 64 * (h % 2)
                    j = h // 2
                    nc.tensor.matmul(
                        Sz[po : po + 64, j, 0:DH],
                        ksb[:, h, :],
                        v1[:, tcc, h, :],
                        start=(cg == 0 and h <= 1),
                        stop=False,
                        skip_group_check=True,
                    )
                    nc.tensor.matmul(
                        Sz[po : po + 64, j, DH : DH + 1],
                        ksb[:, h, :],
                        ones32[:],
                        start=False,
                        stop=(cg == NST * NCH - 1 and h == 7),
                        skip_group_check=True,
                    )
                sz8_new = szsp.tile(
                    [128, 4, DH + 1], bf16, tag="Sz8", name=f"sz8_{cg}"
                )
                nc.scalar.copy(sz8_new[:], Sz[:])
                szh[0] = sz8_new

                # -- normalize -> bf16 o_c --
                oc8 = ocp.tile([C, 8, DH], bf16, tag="oc8", name=f"oc8{cg}")
                nc.vector.tensor_tensor(
                    oc8[:],
                    Ob[:],
                    rden[:].unsqueeze(2).broadcast_to([C, 8, DH]),
                    Alu.mult,
                )

                # -- transpose o_c to feature-major --
                oc_flat = oc8[:].rearrange("p h e -> p (h e)")
                for g in range(4):
                    nc.tensor.matmul(
                        trp[:, g, :],
                        oc_flat[:, g * 128 : (g + 1) * 128],
                        id128[:],
                        is_transpose=True,
                        start=(g == 0),
                        stop=(g == 3),
                        skip_group_check=True,
                    )
                nc.vector.tensor_copy(outT_sb[:, :, tsl], trp[:])

            def emit_outproj_part(st, outT_sb, o3sb, ocs, last):
                t0 = st * ST
                for oc in ocs:
                    po3 = projp.tile([128, ST], f32, tag="proj", name=f"po3_{oc}")
                    for j in range(4):
                        nc.tensor.matmul(
                            po3[:],
                            w2[:, j, oc, :],
                            outT_sb[:, j, :],
                            start=(j == 0),
                            stop=(j == 3),
                        )
                    nc.scalar.copy(o3sb[:, oc, :], po3[:])
                if last:
                    nc.sync.dma_start(
                        outT_d[:, t0 : t0 + ST].rearrange(
                            "(c p) t -> p c t", p=128
                        ),
                        o3sb[:],
                    )

            # ---- software-pipelined emission: chunks of st overlap the
            # projection of st+1 AND the output projection of st-1 ----
            tiles = {0: emit_proj_tiles(0)}
            for fc in range(8):
                emit_fc(tiles[0], fc)
            for tcc in range(NCH):
                emit_v(tiles[0], tcc)
            prev = None  # (st-1, outT_sb, o3sb)
            for st in range(NST):
                T = tiles.pop(st)
                Tn = None
                if st + 1 < NST:
                    Tn = emit_proj_tiles(st + 1)
                    tiles[st + 1] = Tn
                outT_sb = osp.tile(
                    [128, 4, ST], bf16, tag="outT_sb", name=f"osb{st}"
                )
                for tcc in range(NCH):
                    if Tn is not None:
                        emit_fc(Tn, 2 * tcc)
                        emit_fc(Tn, 2 * tcc + 1)
                    emit_chunk(T, st, tcc, outT_sb)
                    if prev is not None:
                        emit_outproj_part(
                            prev[0], prev[1], prev[2],
                            [2 * tcc, 2 * tcc + 1], last=(tcc == NCH - 1),
                        )
                    if Tn is not None and tcc % 2 == 1:
                        emit_v(Tn, tcc - 1)
                        emit_v(Tn, tcc)
                o3sb = o3p.tile([128, 8, ST], bf16, tag="o3sb", name=f"o3sb{st}")
                prev = (st, outT_sb, o3sb)
            emit_outproj_part(prev[0], prev[1], prev[2], list(range(8)), last=True)

    nc.finalize()
    return nc


def _get_nc():
    if "nc" not in _CACHE:
        _CACHE["nc"] = _build_nc()
    return _CACHE["nc"]


def _pack_x(xT):
    """xT: [1024, 4096] f32 -> hi/lo fp8 in [128, 4, 2, L] layout."""
    xr = xT.reshape(4, 2, 128, L).transpose(2, 0, 1, 3)  # [128, 4, 2, L]
    hi = xr.astype(F8)
    lo = (xr - hi.astype(np.float32)).astype(F8)
    return np.ascontiguousarray(hi), np.ascontiguousarray(lo)


def _make_in_maps(x, w_qkv, w_out):
    maskT = np.triu(np.ones((C, C), np.float32))  # maskT[j,i] = 1 if j <= i
    id128 = np.eye(128, dtype=np.float32)
    in_maps = []
    for core in range(8):
        b, g = core // 2, core % 2
        xT = np.ascontiguousarray(x[b].T).astype(np.float32)
        xhi, xlo = _pack_x(xT)
        # q/k weights x32 (fp8 subnormal avoidance), transposed
        W1 = np.concatenate(
            [
                w_qkv[512 * g : 512 * (g + 1)],
                w_qkv[1024 + 512 * g : 1024 + 512 * (g + 1)],
            ],
            axis=0,
        ).T.astype(np.float32)  # [1024 d, 1024 f]
        wqk = (32.0 * W1).reshape(4, 2, 128, 8, 128).transpose(2, 3, 0, 1, 4)
        wqk8 = np.ascontiguousarray(wqk).astype(F8)
        # v weights x32, hi/lo split
        Wv = (32.0 * w_qkv[2048 + 512 * g : 2048 + 512 * (g + 1)].T).astype(
            np.float32
        )  # [1024 d, 512 f]
        Wvr = Wv.reshape(4, 2, 128, FPC).transpose(2, 0, 1, 3)  # [128,4,2,512]
        wvh = Wvr.astype(F8)
        wvl = (Wvr - wvh.astype(np.float32)).astype(F8)
        # output projection weights [512 f, 1024 oc] -> [128, 4, 8, 128]
        W2 = w_out[:, 512 * g : 512 * (g + 1)].T.astype(np.float32)
        w2r = W2.reshape(4, 128, 8, 128).transpose(1, 0, 2, 3)
        in_maps.append(
            {
                "xhi": xhi,
                "xlo": np.ascontiguousarray(xlo),
                "wqk": wqk8,
                "wvh": np.ascontiguousarray(wvh),
                "wvl": np.ascontiguousarray(wvl),
                "w2": np.ascontiguousarray(w2r).astype(BFNP),
                "maskT": maskT,
                "id128": id128.astype(BFNP),
            }
        )
    return in_maps


def _run(inputs, trace=False):
    from concourse.bass_utils import run_bass_kernel_spmd

    nc = _get_nc()
    in_maps = _make_in_maps(inputs["x"], inputs["w_qkv"], inputs["w_out"])
    res = run_bass_kernel_spmd(nc, in_maps, core_ids=list(range(8)), trace=trace)
    out = np.empty((B, L, D), np.float32)
    for b in range(B):
        p0 = res.results[2 * b]["outT"].astype(np.float32)
        p1 = res.results[2 * b + 1]["outT"].astype(np.float32)
        out[b] = (p0 + p1).T
    return out, res


def kernel(x, w_qkv, w_out):
    out, _ = _run({"x": x, "w_qkv": w_qkv, "w_out": w_out})
    return out
